# revision 1
# baseline (speedup 1.0000x reference)
"""Trainium2 Bass kernel for per-aspect 2-layer MLP (embedding-lookup MLP).

Reference computation (B=1024, D=768, H=256, A=20):
    W1 = W1_embs[aspect_ids].reshape(B, D, H)
    out1 = relu(X @batched W1 + b1_embs[aspect_ids])
    logits = out1 @batched W2_embs[aspect_ids].reshape(B, H, 2) + b2

Strategy: there are only A=20 distinct aspects, so group samples by aspect
on the host and turn the per-sample batched matmul into one dense matmul
per aspect.  Shard the 20 aspect-groups across the 8 NeuronCores (3 slots
per core, sorted by group size so slot j has uniform padded size S_j on
every core -> SPMD-uniform program).  Each aspect's W1 (768x256, 786KB)
is then read from HBM exactly once across the whole chip (~16MB total,
~2.4MB per core) instead of once per sample (~800MB).

Per slot the host packs W1 (as 12 [128,128] lhsT chunks), X^T (6 [128,S]
rhs chunks), b1, W2, b2 into a single [128, F] array -> one ~1MB DMA.
On-device, layer 1 is computed transposed: out1T[256,S] = W1^T-chunks
contracted with X^T-chunks (12 accumulating matmuls into two [128,S]
PSUM tiles), so the b1 bias is per-partition and fuses into the
ScalarE activation(Relu, bias) PSUM->SBUF copy.  Layer 2 is 2 tiny
matmuls (lhsT = W2 chunks [128,2]) into a [2,S] PSUM tile + an
Identity-activation with per-partition b2 bias.  logits^T [2,S] tiles
are packed into one SBUF tile and stored with a single DMA.
"""

import numpy as np

N_CORES = 8
PART = 128  # SBUF partitions / PE contraction dim
MAX_N = 512  # max moving free dim for fp32 matmul (one PSUM bank)

_cache: dict = {}


# ───────────────────────── device program ─────────────────────────

def _split_excess_waits(nc):
    """This walrus build rejects >1 sync-wait on one instruction (seen on
    the TileContext tail Drain).  Hoist excess sem waits onto preceding
    NoOps on the same engine — semantically identical (program order)."""
    import concourse.mybir as mybir
    import bass_rust

    n_new = 0
    for f in nc.m.functions:
        for bb in f.blocks:
            insts = bb.instructions
            out = []
            changed = False
            for inst in insts:
                si = inst.sync_info
                if si is not None and si.on_wait and len(si.on_wait) > 1:
                    waits = list(si.on_wait)
                    keep = [w for w in waits if w.wait_reg is not None]
                    movable = [w for w in waits if w.wait_reg is None]
                    while len(keep) < 1 and movable:
                        keep.append(movable.pop())
                    for w in movable:
                        nop = mybir.InstNoOp(
                            name=f"waitsplit_{n_new}", engine=inst.engine,
                            sync_info=bass_rust.SyncInfo(on_wait=[w], on_update=[]))
                        n_new += 1
                        out.append(nop)
                    inst.sync_info = bass_rust.SyncInfo(
                        on_wait=keep, on_update=list(si.on_update))
                    changed = True
                out.append(inst)
            if changed:
                bb.instructions = out
    return n_new


def _build_nc(s_sizes, d, h):
    """Build the SPMD Bass program.  s_sizes: padded group size per slot."""
    import concourse.bass as bass
    import concourse.mybir as mybir
    from concourse.tile import TileContext

    fp32 = mybir.dt.float32
    kd = d // PART           # contraction chunks (6)
    mh = h // PART           # hidden chunks (2)
    w1_f = kd * mh * PART    # 1536 floats of packed W1 per partition

    # per-slot packed layout offsets (floats, within the slot's F columns)
    offs, fs = [], []
    for s in s_sizes:
        o_xt = w1_f
        o_b1 = o_xt + kd * s
        o_w2 = o_b1 + mh
        o_b2 = o_w2 + mh * 2
        f = o_b2 + 1
        f += (-f) % 8  # pad to 32B
        offs.append((o_xt, o_b1, o_w2, o_b2))
        fs.append(f)
    ftot = sum(fs)
    stot = sum(s_sizes)

    nc = bass.Bass()
    IN = nc.dram_tensor("IN", [PART, ftot], fp32, kind="ExternalInput")
    OUT = nc.dram_tensor("OUT", [2, stot], fp32, kind="ExternalOutput")

    with TileContext(nc) as tc:
        with tc.tile_pool(name="inp", bufs=2) as inp_pool, \
             tc.tile_pool(name="o1p", bufs=2 * mh) as o1_pool, \
             tc.tile_pool(name="outp", bufs=1) as out_pool, \
             tc.tile_pool(name="ps1", bufs=2 * mh, space="PSUM") as ps1_pool, \
             tc.tile_pool(name="ps2", bufs=2, space="PSUM") as ps2_pool:

            out_sb = out_pool.tile([2, stot], fp32)

            in_off = 0
            out_off = 0
            for u, s in enumerate(s_sizes):
                o_xt, o_b1, o_w2, o_b2 = offs[u]
                f = fs[u]
                in_t = inp_pool.tile([PART, f], fp32, tag="in_t")
                nc.sync.dma_start(out=in_t[:], in_=IN[:, in_off:in_off + f])

                w1 = in_t[:, 0:w1_f]
                xt = in_t[:, o_xt:o_xt + kd * s]
                b1 = in_t[:, o_b1:o_b1 + mh]
                w2 = in_t[:, o_w2:o_w2 + mh * 2]
                b2 = in_t[0:2, o_b2:o_b2 + 1]

                for n0 in range(0, s, MAX_N):
                    n = min(MAX_N, s - n0)
                    o1s = []
                    for m in range(mh):
                        ps = ps1_pool.tile([PART, n], fp32, tag="ps")
                        for k in range(kd):
                            nc.tensor.matmul(
                                ps[:],
                                w1[:, (k * mh + m) * PART:(k * mh + m + 1) * PART],
                                xt[:, k * s + n0:k * s + n0 + n],
                                start=(k == 0), stop=(k == kd - 1))
                        o1 = o1_pool.tile([PART, n], fp32, tag="o1")
                        nc.scalar.activation(
                            o1[:], ps[:], mybir.ActivationFunctionType.Relu,
                            bias=b1[:, m:m + 1])
                        o1s.append(o1)
                    ps2 = ps2_pool.tile([2, n], fp32, tag="ps2")
                    for m in range(mh):
                        nc.tensor.matmul(
                            ps2[:], w2[:, 2 * m:2 * m + 2], o1s[m][:],
                            start=(m == 0), stop=(m == mh - 1))
                    nc.scalar.activation(
                        out_sb[:, out_off + n0:out_off + n0 + n], ps2[:],
                        mybir.ActivationFunctionType.Identity, bias=b2)
                in_off += f
                out_off += s
            nc.sync.dma_start(out=OUT[:], in_=out_sb[:])

    _split_excess_waits(nc)
    return nc


# ───────────────────────── host side ─────────────────────────

def _install_ntff_hook():
    import sys, types
    if "antenv.axon_hooks" in sys.modules:
        return
    import antenv
    from trn_agent_boot.trn_boot import _ntff_profile_via_ctypes
    mod = types.ModuleType("antenv.axon_hooks")
    hook = _ntff_profile_via_ctypes('/opt/axon/libaxon_pjrt.so')
    mod.get_axon_ntff_profile_hook = lambda: hook
    mod.set_axon_ntff_profile_hook = lambda h: None
    sys.modules["antenv.axon_hooks"] = mod
    antenv.axon_hooks = mod


def _run(X, aspect_ids, W1_embs, b1_embs, W2_embs, b2_embs, trace=False):
    B, D = X.shape
    A, H = b1_embs.shape
    T = b2_embs.shape[1]  # 2 output logits
    assert D % PART == 0 and H % PART == 0 and T == 2
    kd, mh = D // PART, H // PART
    w1_f = kd * mh * PART

    X = np.ascontiguousarray(X, dtype=np.float32)
    W1_embs = np.ascontiguousarray(W1_embs, dtype=np.float32)
    b1_embs = np.ascontiguousarray(b1_embs, dtype=np.float32)
    W2_embs = np.ascontiguousarray(W2_embs, dtype=np.float32)
    b2_embs = np.ascontiguousarray(b2_embs, dtype=np.float32)
    ids = np.asarray(aspect_ids).astype(np.int64)

    # group samples by aspect
    order = np.argsort(ids, kind="stable")
    counts = np.bincount(ids, minlength=A)
    starts = np.concatenate([[0], np.cumsum(counts)])
    rank = np.argsort(-counts, kind="stable")  # aspects, biggest group first

    n_slots = -(-A // N_CORES)  # ceil
    # slot class sizes: class j serves aspects rank[j*8 .. j*8+7]
    s_sizes = []
    for j in range(n_slots):
        cls = rank[j * N_CORES:(j + 1) * N_CORES]
        smax = max(1, int(counts[cls].max()))
        smax += (-smax) % 8
        s_sizes.append(smax)
    key = (tuple(s_sizes), D, H)

    # per-slot packed layout offsets (must match _build_nc)
    offs, fs = [], []
    for s in s_sizes:
        o_xt = w1_f
        o_b1 = o_xt + kd * s
        o_w2 = o_b1 + mh
        o_b2 = o_w2 + mh * 2
        f = o_b2 + 1
        f += (-f) % 8
        offs.append((o_xt, o_b1, o_w2, o_b2))
        fs.append(f)
    ftot = sum(fs)
    stot = sum(s_sizes)

    # pack per-core inputs
    in_maps = []
    unit_info = []  # (core, slot) -> (aspect or -1, idx array, out_off)
    for c in range(N_CORES):
        buf = np.zeros((PART, ftot), dtype=np.float32)
        in_off = 0
        out_off = 0
        for j, s in enumerate(s_sizes):
            r = j * N_CORES + c
            a = int(rank[r]) if r < A else -1
            o_xt, o_b1, o_w2, o_b2 = offs[j]
            if a >= 0:
                n_a = int(counts[a])
                idx = order[starts[a]:starts[a] + n_a]
                # W1 packed: [p, (k*mh+m)*128 + i] = W1[a][(k*128+p)*H + m*128+i]
                buf[:, in_off:in_off + w1_f] = (
                    W1_embs[a].reshape(kd, PART, mh, PART)
                    .transpose(1, 0, 2, 3).reshape(PART, w1_f))
                if n_a > 0:
                    pidx = np.concatenate([idx, np.repeat(idx[:1], s - n_a)])
                    # XT packed: [p, k*s + t] = X[pidx[t], k*128+p]
                    buf[:, in_off + o_xt:in_off + o_xt + kd * s] = (
                        X[pidx].T.reshape(kd, PART, s)
                        .transpose(1, 0, 2).reshape(PART, kd * s))
                buf[:, in_off + o_b1:in_off + o_b1 + mh] = (
                    b1_embs[a].reshape(mh, PART).T)
                buf[:, in_off + o_w2:in_off + o_w2 + mh * 2] = (
                    W2_embs[a].reshape(mh, PART, 2)
                    .transpose(1, 0, 2).reshape(PART, mh * 2))
                buf[0:2, in_off + o_b2] = b2_embs[a]
                unit_info.append((c, j, a, idx, out_off))
            in_off += fs[j]
            out_off += s
        in_maps.append({"IN": buf})

    # build / fetch compiled program
    if key not in _cache:
        _cache[key] = _build_nc(s_sizes, D, H)
    nc = _cache[key]

    if trace:
        _install_ntff_hook()
    from concourse import bass_utils
    bass_utils.upload_artifacts = lambda tmpdir: str(tmpdir)
    res = bass_utils.run_bass_kernel_spmd(
        nc, in_maps, list(range(N_CORES)), trace=trace)

    out = np.zeros((B, T), dtype=np.float32)
    for c, j, a, idx, out_off in unit_info:
        n_a = len(idx)
        if n_a:
            out[idx] = res.results[c]["OUT"][:, out_off:out_off + n_a].T
    return out, res


def kernel(**inputs):
    out, _ = _run(**inputs)
    return out


# revision 7
# speedup vs baseline: 1.0818x; 1.0818x over previous
"""Trainium2 Bass kernel for per-aspect 2-layer MLP (embedding-lookup MLP).

Reference computation (B=1024, D=768, H=256, A=20, T=2):
    W1 = W1_embs[aspect_ids].reshape(B, D, H)
    out1 = relu(X @batched W1 + b1_embs[aspect_ids])
    logits = out1 @batched W2_embs[aspect_ids].reshape(B, H, T) + b2

Strategy: only A=20 distinct aspects exist, so group samples by aspect on
the host and turn the per-sample batched matvec into one dense matmul per
aspect.  Shard the 20 aspect-groups across the 8 NeuronCores (3 slots per
core, groups assigned by size rank so slot j has the same padded size S_j
on every core -> SPMD-uniform program).  Each aspect's W1 (768x256,
786KB) is then read from HBM exactly once across the chip (~16MB total,
~2.4MB per core) instead of once per sample (~800MB).

Device program per slot (S = padded group size, <=128 per chunk):
  - one ~1MB DMA loads the host-packed [128, F] slab: W1 as six [128,256]
    rhs chunks (k-major) + X^T as six [128,S] lhsT chunks.
  - layer 1 on PE: psum[S,256] accumulates 6 matmuls (stationary = X^T
    chunk, moving = W1 chunk, fp32r -> full 1 cycle/row stream rate at
    N=256) + a 7th K=1 matmul (ones[1,S] x b1[1,256]) adding the bias.
  - ScalarE Relu copies psum -> sbuf out1[S,256].
  - layer 2 on DVE: for t in {0,1}, affine_mul_reduce computes
    out1 * w2_t (w2 column DMA-replicated across partitions) with
    accum_out = per-sample sum = logits column; then one tensor_add
    applies b2 (also DMA-replicated).
  - logits live as [S, 2] columns of a persistent [128, 2*n_units] tile;
    one final DMA stores it.

fp32r note: float32r is the TRN2 single-pass fp32 matmul mode (~1.5e-4
relative error vs ~1e-7 for the 2-pass fp32 mode, ~2.7x faster).  Set
VARIANT="fp32" for bit-accurate 2-pass fp32 matmuls.
"""

import numpy as np

N_CORES = 8
PART = 128
VARIANT = "fp32r"  # "fp32r" | "fp32"

_cache: dict = {}


# ───────────────────────── BIR post-pass ─────────────────────────

def _split_excess_waits(nc):
    """This walrus build rejects >1 sync-wait on one instruction (seen on
    the TileContext tail Drain).  Hoist excess sem waits onto preceding
    NoOps on the same engine — semantically identical (program order)."""
    import concourse.mybir as mybir
    import bass_rust

    n_new = 0
    for f in nc.m.functions:
        for bb in f.blocks:
            insts = bb.instructions
            out = []
            changed = False
            for inst in insts:
                si = inst.sync_info
                if si is not None and si.on_wait and len(si.on_wait) > 1:
                    waits = list(si.on_wait)
                    keep = [w for w in waits if w.wait_reg is not None]
                    movable = [w for w in waits if w.wait_reg is None]
                    while len(keep) < 1 and movable:
                        keep.append(movable.pop())
                    for w in movable:
                        nop = mybir.InstNoOp(
                            name=f"waitsplit_{n_new}", engine=inst.engine,
                            sync_info=bass_rust.SyncInfo(on_wait=[w], on_update=[]))
                        n_new += 1
                        out.append(nop)
                    inst.sync_info = bass_rust.SyncInfo(
                        on_wait=keep, on_update=list(si.on_update))
                    changed = True
                out.append(inst)
            if changed:
                bb.instructions = out
    return n_new


# ───────────────────────── device program ─────────────────────────

def _layout(s_sizes, d, h):
    """Packed-slab layout per slot: W1 [128, kd*h] then XT [128, kd*S]."""
    kd = d // PART
    offs, fs = [], []
    for s in s_sizes:
        o_xt = kd * h
        f = o_xt + kd * s
        f += (-f) % 8
        offs.append(o_xt)
        fs.append(f)
    return offs, fs


def _units(s_sizes):
    """(slot, s0, sc) chunks of <=128 samples."""
    us = []
    for j, s in enumerate(s_sizes):
        for s0 in range(0, s, PART):
            us.append((j, s0, min(PART, s - s0)))
    return us


def _build_nc(s_sizes, d, h, variant):
    import concourse.bass as bass
    import concourse.mybir as mybir
    from concourse.tile import TileContext

    fp32 = mybir.dt.float32
    # matmul-operand dtype: float32r = single-pass fp32 PE mode
    mmdt = mybir.dt.float32r if variant == "fp32r" else fp32
    kd, mh = d // PART, h // PART
    offs, fs = _layout(s_sizes, d, h)
    ftot = sum(fs)
    units = _units(s_sizes)
    n_slots = len(s_sizes)

    # SMALLR row layout (matmul-side): per-slot b1[h], then ones[PART]
    smr_per = h
    smr_ones = n_slots * smr_per
    smr_tot = smr_ones + PART
    # SMALLF row layout (DVE-side): per-slot w2 columns [2*h] + b2 [2]
    smf_per = 2 * h + 2
    smf_per += (-smf_per) % 8
    smf_tot = n_slots * smf_per

    T = 2
    nc = bass.Bass()
    IN = nc.dram_tensor("IN", [PART, ftot], mmdt, kind="ExternalInput")
    SMALLR = nc.dram_tensor("SMALLR", [1, smr_tot], mmdt, kind="ExternalInput")
    SMALLF = nc.dram_tensor("SMALLF", [1, smf_tot], fp32, kind="ExternalInput")
    OUT = nc.dram_tensor("OUT", [PART, T * len(units)], fp32,
                         kind="ExternalOutput")

    with TileContext(nc) as tc:
        with tc.tile_pool(name="inp", bufs=3) as inp_pool, \
             tc.tile_pool(name="smallp", bufs=1) as small_pool, \
             tc.tile_pool(name="w2bp", bufs=2) as w2b_pool, \
             tc.tile_pool(name="o1p", bufs=2) as o1_pool, \
             tc.tile_pool(name="scrp", bufs=2) as scr_pool, \
             tc.tile_pool(name="outp", bufs=1) as out_pool, \
             tc.tile_pool(name="ps1", bufs=2, space="PSUM") as ps1_pool:

            out_sb = out_pool.tile([PART, T * len(units)], fp32)
            small_t = small_pool.tile([1, smr_tot], mmdt)
            nc.scalar.dma_start(out=small_t[:], in_=SMALLR[:])

            # prefetch all input slabs (alternate the two HWDGE rings)
            in_ts = []
            in_off = 0
            for j, s in enumerate(s_sizes):
                in_t = inp_pool.tile([PART, fs[j]], mmdt, tag="in_t")
                eng = nc.sync if j % 2 == 0 else nc.scalar
                eng.dma_start(out=in_t[:], in_=IN[:, in_off:in_off + fs[j]])
                in_ts.append(in_t)
                in_off += fs[j]

            # per-slot w2/b2 broadcast tiles (replicate row across partitions)
            w2b_ts = []
            for j in range(n_slots):
                w2b = w2b_pool.tile([PART, 2 * h + 2], fp32, tag="w2b")
                src = SMALLF[0:1, j * smf_per:j * smf_per + 2 * h + 2]
                eng = nc.scalar if j % 2 == 0 else nc.sync
                eng.dma_start(out=w2b[:], in_=src.to_broadcast((PART, 2 * h + 2)))
                w2b_ts.append(w2b)

            for ui, (j, s0, sc) in enumerate(units):
                s = s_sizes[j]
                in_t = in_ts[j]
                o_xt = offs[j]
                w1 = in_t[:, 0:kd * h]
                xt = in_t[:, o_xt:o_xt + kd * s]
                b1row = small_t[0:1, j * smr_per:j * smr_per + h]
                ones = small_t[0:1, smr_ones + 0:smr_ones + sc]
                w2b = w2b_ts[j]

                ps = ps1_pool.tile([sc, h], fp32, tag="ps")
                for k in range(kd):
                    nc.tensor.matmul(
                        ps[:],
                        xt[:, k * s + s0:k * s + s0 + sc],
                        w1[:, k * h:(k + 1) * h],
                        start=(k == 0), stop=False)
                nc.tensor.matmul(
                    ps[:], ones, b1row, start=False, stop=True)

                o1 = o1_pool.tile([PART, h], fp32, tag="o1")
                nc.scalar.activation(
                    o1[:sc], ps[:], mybir.ActivationFunctionType.Relu)

                scr = scr_pool.tile([PART, h], fp32, tag="scr")
                acc = scr_pool.tile([PART, T], fp32, tag="acc")
                for t in range(T):
                    nc.vector.tensor_mul(
                        out=scr[:sc], in0=o1[:sc],
                        in1=w2b[:sc, t * h:(t + 1) * h])
                    nc.vector.reduce_sum(
                        out=acc[:sc, t:t + 1], in_=scr[:sc],
                        axis=mybir.AxisListType.X)
                nc.vector.tensor_add(
                    out=out_sb[:sc, T * ui:T * ui + T],
                    in0=acc[:sc, 0:T],
                    in1=w2b[:sc, 2 * h:2 * h + T])
            nc.sync.dma_start(out=OUT[:], in_=out_sb[:])

    _split_excess_waits(nc)
    return nc


# ───────────────────────── host side ─────────────────────────

def _install_ntff_hook():
    import sys, types
    if "antenv.axon_hooks" in sys.modules:
        return
    import antenv
    from trn_agent_boot.trn_boot import _ntff_profile_via_ctypes
    mod = types.ModuleType("antenv.axon_hooks")
    hook = _ntff_profile_via_ctypes('/opt/axon/libaxon_pjrt.so')
    mod.get_axon_ntff_profile_hook = lambda: hook
    mod.set_axon_ntff_profile_hook = lambda h: None
    sys.modules["antenv.axon_hooks"] = mod
    antenv.axon_hooks = mod


def _run(X, aspect_ids, W1_embs, b1_embs, W2_embs, b2_embs, trace=False):
    B, D = X.shape
    A, H = b1_embs.shape
    T = b2_embs.shape[1]
    assert D % PART == 0 and H % PART == 0 and T == 2
    kd, mh = D // PART, H // PART

    X = np.ascontiguousarray(X, dtype=np.float32)
    W1_embs = np.ascontiguousarray(W1_embs, dtype=np.float32)
    b1_embs = np.ascontiguousarray(b1_embs, dtype=np.float32)
    W2_embs = np.ascontiguousarray(W2_embs, dtype=np.float32)
    b2_embs = np.ascontiguousarray(b2_embs, dtype=np.float32)
    ids = np.asarray(aspect_ids).astype(np.int64)

    order = np.argsort(ids, kind="stable")
    counts = np.bincount(ids, minlength=A)
    starts = np.concatenate([[0], np.cumsum(counts)])
    rank = np.argsort(-counts, kind="stable")

    n_slots = -(-A // N_CORES)
    s_sizes = []
    for j in range(n_slots):
        cls = rank[j * N_CORES:(j + 1) * N_CORES]
        smax = max(1, int(counts[cls].max()))
        smax += (-smax) % 8
        s_sizes.append(smax)

    offs, fs = _layout(s_sizes, D, H)
    ftot = sum(fs)
    units = _units(s_sizes)
    smr_per = H
    smr_ones = n_slots * smr_per
    smr_tot = smr_ones + PART
    smf_per = 2 * H + 2
    smf_per += (-smf_per) % 8
    smf_tot = n_slots * smf_per

    key = (tuple(s_sizes), D, H, VARIANT)
    if key not in _cache:
        _cache[key] = _build_nc(s_sizes, D, H, VARIANT)
    nc = _cache[key]

    w1f = kd * H
    in_maps = []
    scatter = []  # (core, unit_idx, idx_global_rows)
    for c in range(N_CORES):
        buf = np.zeros((PART, ftot), dtype=np.float32)
        smr = np.zeros((1, smr_tot), dtype=np.float32)
        smf = np.zeros((1, smf_tot), dtype=np.float32)
        smr[0, smr_ones:smr_ones + PART] = 1.0
        in_off = 0
        for j, s in enumerate(s_sizes):
            r = j * N_CORES + c
            a = int(rank[r]) if r < A else -1
            if a >= 0:
                n_a = int(counts[a])
                idx = order[starts[a]:starts[a] + n_a]
                buf[:, in_off:in_off + w1f] = (
                    W1_embs[a].reshape(kd, PART, H)
                    .transpose(1, 0, 2).reshape(PART, w1f))
                if n_a > 0:
                    pidx = np.concatenate([idx, np.repeat(idx[:1], s - n_a)])
                    buf[:, in_off + offs[j]:in_off + offs[j] + kd * s] = (
                        X[pidx].T.reshape(kd, PART, s)
                        .transpose(1, 0, 2).reshape(PART, kd * s))
                smr[0, j * smr_per:j * smr_per + H] = b1_embs[a]
                smf[0, j * smf_per:j * smf_per + 2 * H] = (
                    W2_embs[a].reshape(H, T).T.reshape(-1))
                smf[0, j * smf_per + 2 * H:j * smf_per + 2 * H + T] = b2_embs[a]
                for ui, (jj, s0, sc) in enumerate(units):
                    if jj == j and s0 < n_a:
                        scatter.append((c, ui, idx[s0:s0 + sc]))
            in_off += fs[j]
        in_maps.append({"IN": buf, "SMALLR": smr, "SMALLF": smf})

    if trace:
        _install_ntff_hook()
    from concourse import bass_utils
    bass_utils.upload_artifacts = lambda tmpdir: str(tmpdir)
    res = bass_utils.run_bass_kernel_spmd(
        nc, in_maps, list(range(N_CORES)), trace=trace)

    out = np.zeros((B, T), dtype=np.float32)
    for c, ui, idx in scatter:
        out[idx] = res.results[c]["OUT"][:len(idx), T * ui:T * ui + T]
    return out, res


def kernel(**inputs):
    out, _ = _run(**inputs)
    return out


# revision 12
# speedup vs baseline: 1.2590x; 1.1638x over previous
"""Trainium2 Bass kernel for per-aspect 2-layer MLP (embedding-lookup MLP).

Reference computation (B=1024, D=768, H=256, A=20, T=2):
    W1 = W1_embs[aspect_ids].reshape(B, D, H)
    out1 = relu(X @batched W1 + b1_embs[aspect_ids])
    logits = out1 @batched W2_embs[aspect_ids].reshape(B, H, T) + b2

Strategy: only A=20 distinct aspects exist, so group samples by aspect on
the host and turn the per-sample batched matvec into one dense matmul per
aspect.  Shard the 20 aspect-groups across the 8 NeuronCores (3 slots per
core, groups assigned by size rank so slot j has the same padded size S_j
on every core -> SPMD-uniform program).  Each aspect's W1 (768x256,
786KB) is then read from HBM exactly once across the chip (~16MB total,
~2.4MB per core) instead of once per sample (~800MB).

Device program per slot (S = padded group size, <=128 per chunk):
  - one ~1MB DMA loads the host-packed [128, F] slab: W1 as six [128,256]
    rhs chunks (k-major) + X^T as six [128,S] lhsT chunks.
  - layer 1 on PE: psum[S,256] accumulates 6 matmuls (stationary = X^T
    chunk, moving = W1 chunk, fp32r -> full 1 cycle/row stream rate at
    N=256) + a 7th K=1 matmul (ones[1,S] x b1[1,256]) adding the bias.
  - ScalarE Relu copies psum -> sbuf out1[S,256].
  - layer 2 on DVE: for t in {0,1}, affine_mul_reduce computes
    out1 * w2_t (w2 column DMA-replicated across partitions) with
    accum_out = per-sample sum = logits column; then one tensor_add
    applies b2 (also DMA-replicated).
  - logits live as [S, 2] columns of a persistent [128, 2*n_units] tile;
    one final DMA stores it.

fp32r note: float32r is the TRN2 single-pass fp32 matmul mode (~1.5e-4
relative error vs ~1e-7 for the 2-pass fp32 mode, ~2.7x faster).  Set
VARIANT="fp32" for bit-accurate 2-pass fp32 matmuls.
"""

import numpy as np

N_CORES = 8
PART = 128
VARIANT = "fp32r"  # "fp32r" | "fp32"

_cache: dict = {}


# ───────────────────────── BIR post-pass ─────────────────────────

def _split_excess_waits(nc):
    """This walrus build rejects >1 sync-wait on one instruction (seen on
    the TileContext tail Drain).  Hoist excess sem waits onto preceding
    NoOps on the same engine — semantically identical (program order)."""
    import concourse.mybir as mybir
    import bass_rust

    n_new = 0
    for f in nc.m.functions:
        for bb in f.blocks:
            insts = bb.instructions
            out = []
            changed = False
            for inst in insts:
                si = inst.sync_info
                if si is not None and si.on_wait and len(si.on_wait) > 1:
                    waits = list(si.on_wait)
                    keep = [w for w in waits if w.wait_reg is not None]
                    movable = [w for w in waits if w.wait_reg is None]
                    while len(keep) < 1 and movable:
                        keep.append(movable.pop())
                    for w in movable:
                        nop = mybir.InstNoOp(
                            name=f"waitsplit_{n_new}", engine=inst.engine,
                            sync_info=bass_rust.SyncInfo(on_wait=[w], on_update=[]))
                        n_new += 1
                        out.append(nop)
                    inst.sync_info = bass_rust.SyncInfo(
                        on_wait=keep, on_update=list(si.on_update))
                    changed = True
                out.append(inst)
            if changed:
                bb.instructions = out
    return n_new


def _hoist_initial_dmas(nc):
    """Move wait-free input-DMA triggers from the tile body to before the
    program's entry barrier on their issuing engine, so HBM transfers start
    while the engines are still initializing (saves ~6us of startup)."""
    import concourse.mybir as mybir

    f = nc.m.functions[0]
    bbs = list(f.blocks)
    if len(bbs) < 2:
        return 0
    main_bb, body_bb = bbs[0], bbs[1]

    body = body_bb.instructions
    hoisted = {}  # engine -> list[inst]
    remaining = []
    blocked = set()  # engines whose stream hit a non-hoistable inst
    for inst in body:
        eng = inst.engine
        si = inst.sync_info
        is_dma = isinstance(inst, mybir.InstDMACopy)
        waitfree = si is None or not si.on_wait
        if is_dma and waitfree and eng not in blocked:
            hoisted.setdefault(eng, []).append(inst)
        else:
            if eng != mybir.EngineType.Unassigned:
                blocked.add(eng)
            remaining.append(inst)
    if not hoisted:
        return 0

    main = main_bb.instructions
    out = []
    placed = set()
    # insert after the engine's last InstRegisterMove (before its Drain)
    for i, inst in enumerate(main):
        nxt_is_drain = isinstance(inst, mybir.InstDrain)
        if (inst.engine in hoisted and inst.engine not in placed
                and nxt_is_drain):
            out.extend(hoisted[inst.engine])
            placed.add(inst.engine)
        out.append(inst)
    for eng, insts in hoisted.items():
        if eng not in placed:
            out.extend(insts)
    main_bb.instructions = out
    body_bb.instructions = remaining
    return sum(len(v) for v in hoisted.values())


# ───────────────────────── device program ─────────────────────────

def _layout(s_sizes, d, h):
    """Packed-slab layout per slot: W1 [128, kd*h] then XT [128, kd*S]."""
    kd = d // PART
    offs, fs = [], []
    for s in s_sizes:
        o_xt = kd * h
        f = o_xt + kd * s
        f += (-f) % 8
        offs.append(o_xt)
        fs.append(f)
    return offs, fs


def _units(s_sizes):
    """(slot, s0, sc) chunks of <=128 samples."""
    us = []
    for j, s in enumerate(s_sizes):
        for s0 in range(0, s, PART):
            us.append((j, s0, min(PART, s - s0)))
    return us


def _build_nc(s_sizes, d, h, variant):
    import concourse.bass as bass
    import concourse.mybir as mybir
    from concourse.tile import TileContext

    fp32 = mybir.dt.float32
    # matmul-operand dtype: float32r = single-pass fp32 PE mode
    mmdt = mybir.dt.float32r if variant == "fp32r" else fp32
    kd, mh = d // PART, h // PART
    offs, fs = _layout(s_sizes, d, h)
    ftot = sum(fs)
    units = _units(s_sizes)
    n_slots = len(s_sizes)

    # SMALLR row layout (matmul-side): per-slot b1[h], then ones[PART]
    smr_per = h
    smr_ones = n_slots * smr_per
    smr_tot = smr_ones + PART
    # SMALLF row layout (DVE-side): per-slot w2 columns [2*h] + b2 [2]
    smf_per = 2 * h + 2
    smf_per += (-smf_per) % 8
    smf_tot = n_slots * smf_per

    T = 2
    nc = bass.Bass()
    IN = nc.dram_tensor("IN", [PART, ftot], mmdt, kind="ExternalInput")
    SMALLR = nc.dram_tensor("SMALLR", [1, smr_tot], mmdt, kind="ExternalInput")
    SMALLF = nc.dram_tensor("SMALLF", [1, smf_tot], fp32, kind="ExternalInput")
    OUT = nc.dram_tensor("OUT", [PART, T * len(units)], fp32,
                         kind="ExternalOutput")

    with TileContext(nc) as tc:
        with tc.tile_pool(name="inp", bufs=3) as inp_pool, \
             tc.tile_pool(name="smallp", bufs=1) as small_pool, \
             tc.tile_pool(name="w2bp", bufs=2) as w2b_pool, \
             tc.tile_pool(name="o1p", bufs=2) as o1_pool, \
             tc.tile_pool(name="scrp", bufs=2) as scr_pool, \
             tc.tile_pool(name="outp", bufs=1) as out_pool, \
             tc.tile_pool(name="ps1", bufs=2, space="PSUM") as ps1_pool:

            out_sb = out_pool.tile([PART, T * len(units)], fp32)
            small_t = small_pool.tile([1, smr_tot], mmdt)
            nc.scalar.dma_start(out=small_t[:], in_=SMALLR[:])

            # prefetch all input slabs (alternate the two HWDGE rings)
            in_ts = []
            in_off = 0
            for j, s in enumerate(s_sizes):
                in_t = inp_pool.tile([PART, fs[j]], mmdt, tag="in_t")
                eng = nc.sync if j % 2 == 0 else nc.scalar
                eng.dma_start(out=in_t[:], in_=IN[:, in_off:in_off + fs[j]])
                in_ts.append(in_t)
                in_off += fs[j]

            # per-slot w2/b2 broadcast tiles (replicate row across partitions)
            w2b_ts = []
            for j in range(n_slots):
                w2b = w2b_pool.tile([PART, 2 * h + 2], fp32, tag="w2b")
                src = SMALLF[0:1, j * smf_per:j * smf_per + 2 * h + 2]
                eng = nc.scalar if j % 2 == 0 else nc.sync
                eng.dma_start(out=w2b[:], in_=src.to_broadcast((PART, 2 * h + 2)))
                w2b_ts.append(w2b)

            for ui, (j, s0, sc) in enumerate(units):
                s = s_sizes[j]
                in_t = in_ts[j]
                o_xt = offs[j]
                w1 = in_t[:, 0:kd * h]
                xt = in_t[:, o_xt:o_xt + kd * s]
                b1row = small_t[0:1, j * smr_per:j * smr_per + h]
                ones = small_t[0:1, smr_ones + 0:smr_ones + sc]
                w2b = w2b_ts[j]

                ps = ps1_pool.tile([sc, h], fp32, tag="ps")
                for k in range(kd):
                    nc.tensor.matmul(
                        ps[:],
                        xt[:, k * s + s0:k * s + s0 + sc],
                        w1[:, k * h:(k + 1) * h],
                        start=(k == 0), stop=False)
                nc.tensor.matmul(
                    ps[:], ones, b1row, start=False, stop=True)

                o1 = o1_pool.tile([PART, h], fp32, tag="o1")
                nc.scalar.activation(
                    o1[:sc], ps[:], mybir.ActivationFunctionType.Relu)

                scr = scr_pool.tile([PART, T * h], fp32, tag="scr")
                acc = scr_pool.tile([PART, T], fp32, tag="acc")
                for t in range(T):
                    nc.vector.tensor_mul(
                        out=scr[:sc, t * h:(t + 1) * h], in0=o1[:sc],
                        in1=w2b[:sc, t * h:(t + 1) * h])
                    nc.vector.reduce_sum(
                        out=acc[:sc, t:t + 1], in_=scr[:sc, t * h:(t + 1) * h],
                        axis=mybir.AxisListType.X)
                nc.vector.tensor_add(
                    out=out_sb[:sc, T * ui:T * ui + T],
                    in0=acc[:sc, 0:T],
                    in1=w2b[:sc, 2 * h:2 * h + T])
            nc.sync.dma_start(out=OUT[:], in_=out_sb[:])

    _split_excess_waits(nc)
    _hoist_initial_dmas(nc)
    return nc


# ───────────────────────── host side ─────────────────────────

def _install_ntff_hook():
    import sys, types
    if "antenv.axon_hooks" in sys.modules:
        return
    import antenv
    from trn_agent_boot.trn_boot import _ntff_profile_via_ctypes
    mod = types.ModuleType("antenv.axon_hooks")
    hook = _ntff_profile_via_ctypes('/opt/axon/libaxon_pjrt.so')
    mod.get_axon_ntff_profile_hook = lambda: hook
    mod.set_axon_ntff_profile_hook = lambda h: None
    sys.modules["antenv.axon_hooks"] = mod
    antenv.axon_hooks = mod


def _run(X, aspect_ids, W1_embs, b1_embs, W2_embs, b2_embs, trace=False):
    B, D = X.shape
    A, H = b1_embs.shape
    T = b2_embs.shape[1]
    assert D % PART == 0 and H % PART == 0 and T == 2
    kd, mh = D // PART, H // PART

    X = np.ascontiguousarray(X, dtype=np.float32)
    W1_embs = np.ascontiguousarray(W1_embs, dtype=np.float32)
    b1_embs = np.ascontiguousarray(b1_embs, dtype=np.float32)
    W2_embs = np.ascontiguousarray(W2_embs, dtype=np.float32)
    b2_embs = np.ascontiguousarray(b2_embs, dtype=np.float32)
    ids = np.asarray(aspect_ids).astype(np.int64)

    order = np.argsort(ids, kind="stable")
    counts = np.bincount(ids, minlength=A)
    starts = np.concatenate([[0], np.cumsum(counts)])
    rank = np.argsort(-counts, kind="stable")

    n_slots = -(-A // N_CORES)
    s_sizes = []
    for j in range(n_slots):
        cls = rank[j * N_CORES:(j + 1) * N_CORES]
        smax = max(1, int(counts[cls].max()))
        smax += (-smax) % 8
        s_sizes.append(smax)

    offs, fs = _layout(s_sizes, D, H)
    ftot = sum(fs)
    units = _units(s_sizes)
    smr_per = H
    smr_ones = n_slots * smr_per
    smr_tot = smr_ones + PART
    smf_per = 2 * H + 2
    smf_per += (-smf_per) % 8
    smf_tot = n_slots * smf_per

    key = (tuple(s_sizes), D, H, VARIANT)
    if key not in _cache:
        _cache[key] = _build_nc(s_sizes, D, H, VARIANT)
    nc = _cache[key]

    w1f = kd * H
    in_maps = []
    scatter = []  # (core, unit_idx, idx_global_rows)
    for c in range(N_CORES):
        buf = np.zeros((PART, ftot), dtype=np.float32)
        smr = np.zeros((1, smr_tot), dtype=np.float32)
        smf = np.zeros((1, smf_tot), dtype=np.float32)
        smr[0, smr_ones:smr_ones + PART] = 1.0
        in_off = 0
        for j, s in enumerate(s_sizes):
            r = j * N_CORES + c
            a = int(rank[r]) if r < A else -1
            if a >= 0:
                n_a = int(counts[a])
                idx = order[starts[a]:starts[a] + n_a]
                buf[:, in_off:in_off + w1f] = (
                    W1_embs[a].reshape(kd, PART, H)
                    .transpose(1, 0, 2).reshape(PART, w1f))
                if n_a > 0:
                    pidx = np.concatenate([idx, np.repeat(idx[:1], s - n_a)])
                    buf[:, in_off + offs[j]:in_off + offs[j] + kd * s] = (
                        X[pidx].T.reshape(kd, PART, s)
                        .transpose(1, 0, 2).reshape(PART, kd * s))
                smr[0, j * smr_per:j * smr_per + H] = b1_embs[a]
                smf[0, j * smf_per:j * smf_per + 2 * H] = (
                    W2_embs[a].reshape(H, T).T.reshape(-1))
                smf[0, j * smf_per + 2 * H:j * smf_per + 2 * H + T] = b2_embs[a]
                for ui, (jj, s0, sc) in enumerate(units):
                    if jj == j and s0 < n_a:
                        scatter.append((c, ui, idx[s0:s0 + sc]))
            in_off += fs[j]
        in_maps.append({"IN": buf, "SMALLR": smr, "SMALLF": smf})

    if trace:
        _install_ntff_hook()
    from concourse import bass_utils
    bass_utils.upload_artifacts = lambda tmpdir: str(tmpdir)
    res = bass_utils.run_bass_kernel_spmd(
        nc, in_maps, list(range(N_CORES)), trace=trace)

    out = np.zeros((B, T), dtype=np.float32)
    for c, ui, idx in scatter:
        out[idx] = res.results[c]["OUT"][:len(idx), T * ui:T * ui + T]
    return out, res


def kernel(**inputs):
    out, _ = _run(**inputs)
    return out


# revision 19
# speedup vs baseline: 1.3113x; 1.0415x over previous
"""Trainium2 Bass kernel for per-aspect 2-layer MLP (embedding-lookup MLP).

Reference computation (B=1024, D=768, H=256, A=20, T=2):
    W1 = W1_embs[aspect_ids].reshape(B, D, H)
    out1 = relu(X @batched W1 + b1_embs[aspect_ids])
    logits = out1 @batched W2_embs[aspect_ids].reshape(B, H, T) + b2

Strategy: only A=20 distinct aspects exist, so group samples by aspect on
the host and turn the per-sample batched matvec into one dense matmul per
aspect.  Shard the 20 aspect-groups across the 8 NeuronCores (3 slots per
core, groups assigned by size rank so slot j has the same padded size S_j
on every core -> SPMD-uniform program).  Each aspect's W1 (768x256,
786KB) is then read from HBM exactly once across the chip (~16MB total,
~2.4MB per core) instead of once per sample (~800MB).

Device program per slot (S = padded group size, <=128 per chunk):
  - one ~1MB DMA loads the host-packed [128, F] slab: W1 as six [128,256]
    rhs chunks (k-major) + X^T as six [128,S] lhsT chunks.
  - layer 1 on PE: psum[S,256] accumulates 6 matmuls (stationary = X^T
    chunk, moving = W1 chunk, fp32r -> full 1 cycle/row stream rate at
    N=256) + a 7th K=1 matmul (ones[1,S] x b1[1,256]) adding the bias.
  - ScalarE Relu copies psum -> sbuf out1[S,256].
  - layer 2 on DVE: for t in {0,1}, affine_mul_reduce computes
    out1 * w2_t (w2 column DMA-replicated across partitions) with
    accum_out = per-sample sum = logits column; then one tensor_add
    applies b2 (also DMA-replicated).
  - logits live as [S, 2] columns of a persistent [128, 2*n_units] tile;
    one final DMA stores it.

fp32r note: float32r is the TRN2 single-pass fp32 matmul mode (~1.5e-4
relative error vs ~1e-7 for the 2-pass fp32 mode, ~2.7x faster).  Set
VARIANT="fp32" for bit-accurate 2-pass fp32 matmuls.
"""

import numpy as np

N_CORES = 8
PART = 128
VARIANT = "fp32r"  # "fp32r" | "fp32"

_cache: dict = {}


# ───────────────────────── BIR post-pass ─────────────────────────

def _split_excess_waits(nc):
    """This walrus build rejects >1 sync-wait on one instruction (seen on
    the TileContext tail Drain).  Hoist excess sem waits onto preceding
    NoOps on the same engine — semantically identical (program order)."""
    import concourse.mybir as mybir
    import bass_rust

    n_new = 0
    for f in nc.m.functions:
        for bb in f.blocks:
            insts = bb.instructions
            out = []
            changed = False
            for inst in insts:
                si = inst.sync_info
                if si is not None and si.on_wait and len(si.on_wait) > 1:
                    waits = list(si.on_wait)
                    keep = [w for w in waits if w.wait_reg is not None]
                    movable = [w for w in waits if w.wait_reg is None]
                    while len(keep) < 1 and movable:
                        keep.append(movable.pop())
                    for w in movable:
                        nop = mybir.InstNoOp(
                            name=f"waitsplit_{n_new}", engine=inst.engine,
                            sync_info=bass_rust.SyncInfo(on_wait=[w], on_update=[]))
                        n_new += 1
                        out.append(nop)
                    inst.sync_info = bass_rust.SyncInfo(
                        on_wait=keep, on_update=list(si.on_update))
                    changed = True
                out.append(inst)
            if changed:
                bb.instructions = out
    return n_new


def _hoist_initial_dmas(nc):
    """Move wait-free input-DMA triggers from the tile body to before the
    program's entry barrier on their issuing engine, so HBM transfers start
    while the engines are still initializing (saves ~6us of startup)."""
    import concourse.mybir as mybir

    f = nc.m.functions[0]
    bbs = list(f.blocks)
    if len(bbs) < 2:
        return 0
    main_bb, body_bb = bbs[0], bbs[1]

    body = body_bb.instructions
    hoisted = {}  # engine -> list[inst]
    remaining = []
    blocked = set()  # engines whose stream hit a non-hoistable inst
    for inst in body:
        eng = inst.engine
        si = inst.sync_info
        is_dma = isinstance(inst, mybir.InstDMACopy)
        waitfree = si is None or not si.on_wait
        if is_dma and waitfree and eng not in blocked:
            hoisted.setdefault(eng, []).append(inst)
        else:
            if eng != mybir.EngineType.Unassigned:
                blocked.add(eng)
            remaining.append(inst)
    if not hoisted:
        return 0

    main = main_bb.instructions
    out = []
    placed = set()
    # insert after the engine's last InstRegisterMove (before its Drain)
    for i, inst in enumerate(main):
        nxt_is_drain = isinstance(inst, mybir.InstDrain)
        if (inst.engine in hoisted and inst.engine not in placed
                and nxt_is_drain):
            out.extend(hoisted[inst.engine])
            placed.add(inst.engine)
        out.append(inst)
    for eng, insts in hoisted.items():
        if eng not in placed:
            out.extend(insts)
    main_bb.instructions = out
    body_bb.instructions = remaining
    return sum(len(v) for v in hoisted.values())


# ───────────────────────── device program ─────────────────────────

def _layout(s_sizes, d, h):
    """Packed-slab layout per slot: W1 [128, kd*h] then XT [128, kd*S]."""
    kd = d // PART
    offs, fs = [], []
    for s in s_sizes:
        o_xt = kd * h
        f = o_xt + kd * s
        f += (-f) % 8
        offs.append(o_xt)
        fs.append(f)
    return offs, fs


def _units(s_sizes):
    """(slot, s0, sc) chunks of <=128 samples."""
    us = []
    for j, s in enumerate(s_sizes):
        for s0 in range(0, s, PART):
            us.append((j, s0, min(PART, s - s0)))
    return us


def _build_nc(s_sizes, d, h, variant):
    import concourse.bass as bass
    import concourse.mybir as mybir
    from concourse.tile import TileContext

    fp32 = mybir.dt.float32
    # matmul-operand dtype: float32r = single-pass fp32 PE mode
    mmdt = mybir.dt.float32r if variant == "fp32r" else fp32
    kd, mh = d // PART, h // PART
    offs, fs = _layout(s_sizes, d, h)
    ftot = sum(fs)
    units = _units(s_sizes)
    n_slots = len(s_sizes)

    # SMALLR row layout: per-slot [b1[h] | w2col0,b2_0 [h+1] | w2col1,b2_1
    # [h+1]], then ones[PART]
    T = 2
    smr_per = h + T * (h + 2)
    smr_per += (-smr_per) % 8
    smr_ones = n_slots * smr_per
    smr_tot = smr_ones + PART

    nc = bass.Bass()
    IN = nc.dram_tensor("IN", [PART, ftot], mmdt, kind="ExternalInput")
    SMALLR = nc.dram_tensor("SMALLR", [1, smr_tot], mmdt, kind="ExternalInput")
    OUT = nc.dram_tensor("OUT", [PART, T * len(units)], fp32,
                         kind="ExternalOutput")

    with TileContext(nc) as tc:
        with tc.tile_pool(name="inp", bufs=3) as inp_pool, \
             tc.tile_pool(name="smallp", bufs=1) as small_pool, \
             tc.tile_pool(name="o1p", bufs=2) as o1_pool, \
             tc.tile_pool(name="scrp", bufs=2) as scr_pool, \
             tc.tile_pool(name="outp", bufs=1) as out_pool, \
             tc.tile_pool(name="ps1", bufs=2, space="PSUM") as ps1_pool, \
             tc.tile_pool(name="psw", bufs=T * n_slots, space="PSUM") as psw_pool:

            out_sb = out_pool.tile([PART, T * len(units)], fp32)
            small_t = small_pool.tile([1, smr_tot], mmdt)
            nc.scalar.dma_start(out=small_t[:], in_=SMALLR[:])

            # prefetch all input slabs (alternate the two HWDGE rings)
            in_ts = []
            in_off = 0
            for j, s in enumerate(s_sizes):
                in_t = inp_pool.tile([PART, fs[j]], mmdt, tag="in_t")
                eng = nc.sync if j % 2 == 0 else nc.scalar
                eng.dma_start(out=in_t[:], in_=IN[:, in_off:in_off + fs[j]])
                in_ts.append(in_t)
                in_off += fs[j]

            ones_full = small_t[0:1, smr_ones:smr_ones + PART]

            # replicate each slot's w2 column (+b2) across all partitions on
            # the PE: psum[p, f] = ones[p] * w2row[f]; DVE then reads PSUM.
            w2ps = []
            for j in range(n_slots):
                for t in range(T):
                    wp = psw_pool.tile([PART, h + 2], fp32, tag="w2ps")
                    src = small_t[0:1, j * smr_per + h + t * (h + 2):
                                  j * smr_per + h + (t + 1) * (h + 2)]
                    nc.tensor.matmul(wp[:], ones_full, src,
                                     start=True, stop=True)
                    w2ps.append(wp)

            for ui, (j, s0, sc) in enumerate(units):
                s = s_sizes[j]
                in_t = in_ts[j]
                o_xt = offs[j]
                w1 = in_t[:, 0:kd * h]
                xt = in_t[:, o_xt:o_xt + kd * s]
                b1row = small_t[0:1, j * smr_per:j * smr_per + h]
                ones = small_t[0:1, smr_ones + 0:smr_ones + sc]

                ps = ps1_pool.tile([sc, h], fp32, tag="ps")
                for k in range(kd):
                    nc.tensor.matmul(
                        ps[:],
                        xt[:, k * s + s0:k * s + s0 + sc],
                        w1[:, k * h:(k + 1) * h],
                        start=(k == 0), stop=False)
                nc.tensor.matmul(
                    ps[:], ones, b1row, start=False, stop=True)

                o1 = o1_pool.tile([PART, h], fp32, tag="o1")
                nc.scalar.activation(
                    o1[:sc], ps[:], mybir.ActivationFunctionType.Relu)

                scr = scr_pool.tile([PART, T * h], fp32, tag="scr")
                acc = scr_pool.tile([PART, T], fp32, tag="acc")
                for t in range(T):
                    wp = w2ps[j * T + t]
                    nc.vector.tensor_mul(
                        out=scr[:sc, t * h:(t + 1) * h], in0=o1[:sc],
                        in1=wp[:sc, 0:h])
                    nc.vector.reduce_sum(
                        out=acc[:sc, t:t + 1], in_=scr[:sc, t * h:(t + 1) * h],
                        axis=mybir.AxisListType.X)
                    nc.vector.tensor_add(
                        out=out_sb[:sc, T * ui + t:T * ui + t + 1],
                        in0=acc[:sc, t:t + 1],
                        in1=wp[:sc, h:h + 1])
            nc.sync.dma_start(out=OUT[:], in_=out_sb[:])

    _split_excess_waits(nc)
    _hoist_initial_dmas(nc)
    return nc


# ───────────────────────── host side ─────────────────────────

def _install_ntff_hook():
    import sys, types
    if "antenv.axon_hooks" in sys.modules:
        return
    import antenv
    from trn_agent_boot.trn_boot import _ntff_profile_via_ctypes
    mod = types.ModuleType("antenv.axon_hooks")
    hook = _ntff_profile_via_ctypes('/opt/axon/libaxon_pjrt.so')
    mod.get_axon_ntff_profile_hook = lambda: hook
    mod.set_axon_ntff_profile_hook = lambda h: None
    sys.modules["antenv.axon_hooks"] = mod
    antenv.axon_hooks = mod


def _run(X, aspect_ids, W1_embs, b1_embs, W2_embs, b2_embs, trace=False):
    B, D = X.shape
    A, H = b1_embs.shape
    T = b2_embs.shape[1]
    assert D % PART == 0 and H % PART == 0 and T == 2
    kd, mh = D // PART, H // PART

    X = np.ascontiguousarray(X, dtype=np.float32)
    W1_embs = np.ascontiguousarray(W1_embs, dtype=np.float32)
    b1_embs = np.ascontiguousarray(b1_embs, dtype=np.float32)
    W2_embs = np.ascontiguousarray(W2_embs, dtype=np.float32)
    b2_embs = np.ascontiguousarray(b2_embs, dtype=np.float32)
    ids = np.asarray(aspect_ids).astype(np.int64)

    order = np.argsort(ids, kind="stable")
    counts = np.bincount(ids, minlength=A)
    starts = np.concatenate([[0], np.cumsum(counts)])
    rank = np.argsort(-counts, kind="stable")

    n_slots = -(-A // N_CORES)
    s_sizes = []
    for j in range(n_slots):
        cls = rank[j * N_CORES:(j + 1) * N_CORES]
        smax = max(1, int(counts[cls].max()))
        smax += (-smax) % 8
        s_sizes.append(smax)

    offs, fs = _layout(s_sizes, D, H)
    ftot = sum(fs)
    units = _units(s_sizes)
    smr_per = H + T * (H + 2)
    smr_per += (-smr_per) % 8
    smr_ones = n_slots * smr_per
    smr_tot = smr_ones + PART

    key = (tuple(s_sizes), D, H, VARIANT)
    if key not in _cache:
        _cache[key] = _build_nc(s_sizes, D, H, VARIANT)
    nc = _cache[key]

    w1f = kd * H
    in_maps = []
    scatter = []  # (core, unit_idx, idx_global_rows)
    for c in range(N_CORES):
        buf = np.zeros((PART, ftot), dtype=np.float32)
        smr = np.zeros((1, smr_tot), dtype=np.float32)
        smr[0, smr_ones:smr_ones + PART] = 1.0
        in_off = 0
        for j, s in enumerate(s_sizes):
            r = j * N_CORES + c
            a = int(rank[r]) if r < A else -1
            if a >= 0:
                n_a = int(counts[a])
                idx = order[starts[a]:starts[a] + n_a]
                buf[:, in_off:in_off + w1f] = (
                    W1_embs[a].reshape(kd, PART, H)
                    .transpose(1, 0, 2).reshape(PART, w1f))
                if n_a > 0:
                    pidx = np.concatenate([idx, np.repeat(idx[:1], s - n_a)])
                    buf[:, in_off + offs[j]:in_off + offs[j] + kd * s] = (
                        X[pidx].T.reshape(kd, PART, s)
                        .transpose(1, 0, 2).reshape(PART, kd * s))
                smr[0, j * smr_per:j * smr_per + H] = b1_embs[a]
                w2c = W2_embs[a].reshape(H, T)
                for t in range(T):
                    base = j * smr_per + H + t * (H + 2)
                    smr[0, base:base + H] = w2c[:, t]
                    smr[0, base + H] = b2_embs[a][t]
                for ui, (jj, s0, sc) in enumerate(units):
                    if jj == j and s0 < n_a:
                        scatter.append((c, ui, idx[s0:s0 + sc]))
            in_off += fs[j]
        in_maps.append({"IN": buf, "SMALLR": smr})

    if trace:
        _install_ntff_hook()
    from concourse import bass_utils
    bass_utils.upload_artifacts = lambda tmpdir: str(tmpdir)
    res = bass_utils.run_bass_kernel_spmd(
        nc, in_maps, list(range(N_CORES)), trace=trace)

    out = np.zeros((B, T), dtype=np.float32)
    for c, ui, idx in scatter:
        out[idx] = res.results[c]["OUT"][:len(idx), T * ui:T * ui + T]
    return out, res


def kernel(**inputs):
    out, _ = _run(**inputs)
    return out


# revision 21
# speedup vs baseline: 1.4092x; 1.0746x over previous
"""Trainium2 Bass kernel for per-aspect 2-layer MLP (embedding-lookup MLP).

Reference computation (B=1024, D=768, H=256, A=20, T=2):
    W1 = W1_embs[aspect_ids].reshape(B, D, H)
    out1 = relu(X @batched W1 + b1_embs[aspect_ids])
    logits = out1 @batched W2_embs[aspect_ids].reshape(B, H, T) + b2

Strategy: only A=20 distinct aspects exist, so group samples by aspect on
the host and turn the per-sample batched matvec into one dense matmul per
aspect.  Shard the 20 aspect-groups across the 8 NeuronCores (3 slots per
core, groups assigned by size rank so slot j has the same padded size S_j
on every core -> SPMD-uniform program).  Each aspect's W1 (768x256,
786KB) is then read from HBM exactly once across the chip (~16MB total,
~2.4MB per core) instead of once per sample (~800MB).

Device program per slot (S = padded group size, <=128 per chunk):
  - one ~1MB DMA loads the host-packed [128, F] slab: W1 as six [128,256]
    rhs chunks (k-major) + X^T as six [128,S] lhsT chunks.
  - layer 1 on PE: psum[S,256] accumulates 6 matmuls (stationary = X^T
    chunk, moving = W1 chunk, fp32r -> full 1 cycle/row stream rate at
    N=256) + a 7th K=1 matmul (ones[1,S] x b1[1,256]) adding the bias.
  - ScalarE Relu copies psum -> sbuf out1[S,256].
  - layer 2 on DVE: for t in {0,1}, affine_mul_reduce computes
    out1 * w2_t (w2 column DMA-replicated across partitions) with
    accum_out = per-sample sum = logits column; then one tensor_add
    applies b2 (also DMA-replicated).
  - logits live as [S, 2] columns of a persistent [128, 2*n_units] tile;
    one final DMA stores it.

fp32r note: float32r is the TRN2 single-pass fp32 matmul mode (~1.5e-4
relative error vs ~1e-7 for the 2-pass fp32 mode, ~2.7x faster).  Set
VARIANT="fp32" for bit-accurate 2-pass fp32 matmuls.
"""

import numpy as np

N_CORES = 8
PART = 128
VARIANT = "fp32r"  # "fp32r" | "fp32"

_cache: dict = {}


# ───────────────────────── BIR post-pass ─────────────────────────

def _split_excess_waits(nc):
    """This walrus build rejects >1 sync-wait on one instruction (seen on
    the TileContext tail Drain).  Hoist excess sem waits onto preceding
    NoOps on the same engine — semantically identical (program order)."""
    import concourse.mybir as mybir
    import bass_rust

    n_new = 0
    for f in nc.m.functions:
        for bb in f.blocks:
            insts = bb.instructions
            out = []
            changed = False
            for inst in insts:
                si = inst.sync_info
                if si is not None and si.on_wait and len(si.on_wait) > 1:
                    waits = list(si.on_wait)
                    keep = [w for w in waits if w.wait_reg is not None]
                    movable = [w for w in waits if w.wait_reg is None]
                    while len(keep) < 1 and movable:
                        keep.append(movable.pop())
                    for w in movable:
                        nop = mybir.InstNoOp(
                            name=f"waitsplit_{n_new}", engine=inst.engine,
                            sync_info=bass_rust.SyncInfo(on_wait=[w], on_update=[]))
                        n_new += 1
                        out.append(nop)
                    inst.sync_info = bass_rust.SyncInfo(
                        on_wait=keep, on_update=list(si.on_update))
                    changed = True
                out.append(inst)
            if changed:
                bb.instructions = out
    return n_new


def _hoist_initial_dmas(nc):
    """Move wait-free input-DMA triggers from the tile body to before the
    program's entry barrier on their issuing engine, so HBM transfers start
    while the engines are still initializing (saves ~6us of startup)."""
    import concourse.mybir as mybir

    f = nc.m.functions[0]
    bbs = list(f.blocks)
    if len(bbs) < 2:
        return 0
    main_bb, body_bb = bbs[0], bbs[1]

    body = body_bb.instructions
    hoisted = {}  # engine -> list[inst]
    remaining = []
    blocked = set()  # engines whose stream hit a non-hoistable inst
    for inst in body:
        eng = inst.engine
        si = inst.sync_info
        is_dma = isinstance(inst, mybir.InstDMACopy)
        waitfree = si is None or not si.on_wait
        if is_dma and waitfree and eng not in blocked:
            hoisted.setdefault(eng, []).append(inst)
        else:
            if eng != mybir.EngineType.Unassigned:
                blocked.add(eng)
            remaining.append(inst)
    if not hoisted:
        return 0

    main = main_bb.instructions
    out = []
    placed = set()
    # insert after the engine's last InstRegisterMove (before its Drain)
    for i, inst in enumerate(main):
        nxt_is_drain = isinstance(inst, mybir.InstDrain)
        if (inst.engine in hoisted and inst.engine not in placed
                and nxt_is_drain):
            out.extend(hoisted[inst.engine])
            placed.add(inst.engine)
        out.append(inst)
    for eng, insts in hoisted.items():
        if eng not in placed:
            out.extend(insts)
    main_bb.instructions = out
    body_bb.instructions = remaining
    return sum(len(v) for v in hoisted.values())


# ───────────────────────── device program ─────────────────────────

def _layout(s_sizes, d, h):
    """Half-slab layout: per slot TWO slabs (k-groups), each
    [128, (kd/2)*h + (kd/2)*S]: W1 chunks then XT chunks."""
    kd = d // PART
    kh = kd // 2
    offs, fs = [], []
    for s in s_sizes:
        o_xt = kh * h
        f = o_xt + kh * s
        f += (-f) % 8
        offs.append(o_xt)
        fs.append(f)
    return offs, fs


def _units(s_sizes):
    """(slot, s0, sc) chunks of <=128 samples."""
    us = []
    for j, s in enumerate(s_sizes):
        for s0 in range(0, s, PART):
            us.append((j, s0, min(PART, s - s0)))
    return us


def _build_nc(s_sizes, d, h, variant):
    import concourse.bass as bass
    import concourse.mybir as mybir
    from concourse.tile import TileContext

    fp32 = mybir.dt.float32
    # matmul-operand dtype: float32r = single-pass fp32 PE mode
    mmdt = mybir.dt.float32r if variant == "fp32r" else fp32
    kd, mh = d // PART, h // PART
    offs, fs = _layout(s_sizes, d, h)
    ftot = 2 * sum(fs)
    units = _units(s_sizes)
    n_slots = len(s_sizes)

    # SMALLR row layout: per-slot [b1[h] | w2col0,b2_0 [h+1] | w2col1,b2_1
    # [h+1]], then ones[PART]
    T = 2
    smr_per = h + T * (h + 2)
    smr_per += (-smr_per) % 8
    smr_ones = n_slots * smr_per
    smr_tot = smr_ones + PART

    nc = bass.Bass()
    IN = nc.dram_tensor("IN", [PART, ftot], mmdt, kind="ExternalInput")
    SMALLR = nc.dram_tensor("SMALLR", [1, smr_tot], mmdt, kind="ExternalInput")
    OUT = nc.dram_tensor("OUT", [PART, T * len(units)], fp32,
                         kind="ExternalOutput")

    with TileContext(nc) as tc:
        with tc.tile_pool(name="inp", bufs=6) as inp_pool, \
             tc.tile_pool(name="smallp", bufs=1) as small_pool, \
             tc.tile_pool(name="o1p", bufs=2) as o1_pool, \
             tc.tile_pool(name="scrp", bufs=2) as scr_pool, \
             tc.tile_pool(name="outp", bufs=1) as out_pool, \
             tc.tile_pool(name="ps1", bufs=2, space="PSUM") as ps1_pool, \
             tc.tile_pool(name="psw", bufs=T * n_slots, space="PSUM") as psw_pool:

            out_sb = out_pool.tile([PART, T * len(units)], fp32)
            small_t = small_pool.tile([1, smr_tot], mmdt)
            nc.scalar.dma_start(out=small_t[:], in_=SMALLR[:])

            # prefetch all input half-slabs on ONE ring in consumption
            # order (per-ring FIFO => data lands exactly in compute order;
            # a second ring would only steal bandwidth from the head).
            in_ts = []
            in_off = 0
            for j, s in enumerate(s_sizes):
                pair = []
                for g in range(2):
                    in_t = inp_pool.tile([PART, fs[j]], mmdt, tag="in_t")
                    nc.sync.dma_start(
                        out=in_t[:], in_=IN[:, in_off:in_off + fs[j]])
                    pair.append(in_t)
                    in_off += fs[j]
                in_ts.append(pair)

            ones_full = small_t[0:1, smr_ones:smr_ones + PART]

            # replicate each slot's w2 column (+b2) across all partitions on
            # the PE: psum[p, f] = ones[p] * w2row[f]; DVE then reads PSUM.
            w2ps = []
            for j in range(n_slots):
                for t in range(T):
                    wp = psw_pool.tile([PART, h + 2], fp32, tag="w2ps")
                    src = small_t[0:1, j * smr_per + h + t * (h + 2):
                                  j * smr_per + h + (t + 1) * (h + 2)]
                    nc.tensor.matmul(wp[:], ones_full, src,
                                     start=True, stop=True)
                    w2ps.append(wp)

            for ui, (j, s0, sc) in enumerate(units):
                s = s_sizes[j]
                o_xt = offs[j]
                kh = kd // 2
                b1row = small_t[0:1, j * smr_per:j * smr_per + h]
                ones = small_t[0:1, smr_ones + 0:smr_ones + sc]

                ps = ps1_pool.tile([sc, h], fp32, tag="ps")
                for k in range(kd):
                    in_t = in_ts[j][k // kh]
                    kk = k % kh
                    nc.tensor.matmul(
                        ps[:],
                        in_t[:, o_xt + kk * s + s0:o_xt + kk * s + s0 + sc],
                        in_t[:, kk * h:(kk + 1) * h],
                        start=(k == 0), stop=False)
                nc.tensor.matmul(
                    ps[:], ones, b1row, start=False, stop=True)

                o1 = o1_pool.tile([PART, h], fp32, tag="o1")
                nc.scalar.activation(
                    o1[:sc], ps[:], mybir.ActivationFunctionType.Relu)

                scr = scr_pool.tile([PART, T * h], fp32, tag="scr")
                acc = scr_pool.tile([PART, T], fp32, tag="acc")
                for t in range(T):
                    wp = w2ps[j * T + t]
                    nc.vector.tensor_mul(
                        out=scr[:sc, t * h:(t + 1) * h], in0=o1[:sc],
                        in1=wp[:sc, 0:h])
                    nc.vector.reduce_sum(
                        out=acc[:sc, t:t + 1], in_=scr[:sc, t * h:(t + 1) * h],
                        axis=mybir.AxisListType.X)
                    nc.vector.tensor_add(
                        out=out_sb[:sc, T * ui + t:T * ui + t + 1],
                        in0=acc[:sc, t:t + 1],
                        in1=wp[:sc, h:h + 1])
            nc.scalar.dma_start(out=OUT[:], in_=out_sb[:])

    _split_excess_waits(nc)
    _hoist_initial_dmas(nc)
    return nc


# ───────────────────────── host side ─────────────────────────

def _install_ntff_hook():
    import sys, types
    if "antenv.axon_hooks" in sys.modules:
        return
    import antenv
    from trn_agent_boot.trn_boot import _ntff_profile_via_ctypes
    mod = types.ModuleType("antenv.axon_hooks")
    hook = _ntff_profile_via_ctypes('/opt/axon/libaxon_pjrt.so')
    mod.get_axon_ntff_profile_hook = lambda: hook
    mod.set_axon_ntff_profile_hook = lambda h: None
    sys.modules["antenv.axon_hooks"] = mod
    antenv.axon_hooks = mod


def _run(X, aspect_ids, W1_embs, b1_embs, W2_embs, b2_embs, trace=False):
    B, D = X.shape
    A, H = b1_embs.shape
    T = b2_embs.shape[1]
    assert D % PART == 0 and H % PART == 0 and T == 2
    kd, mh = D // PART, H // PART

    X = np.ascontiguousarray(X, dtype=np.float32)
    W1_embs = np.ascontiguousarray(W1_embs, dtype=np.float32)
    b1_embs = np.ascontiguousarray(b1_embs, dtype=np.float32)
    W2_embs = np.ascontiguousarray(W2_embs, dtype=np.float32)
    b2_embs = np.ascontiguousarray(b2_embs, dtype=np.float32)
    ids = np.asarray(aspect_ids).astype(np.int64)

    order = np.argsort(ids, kind="stable")
    counts = np.bincount(ids, minlength=A)
    starts = np.concatenate([[0], np.cumsum(counts)])
    rank = np.argsort(-counts, kind="stable")

    n_slots = -(-A // N_CORES)
    s_sizes = []
    for j in range(n_slots):
        cls = rank[j * N_CORES:(j + 1) * N_CORES]
        smax = max(1, int(counts[cls].max()))
        smax += (-smax) % 8
        s_sizes.append(smax)

    offs, fs = _layout(s_sizes, D, H)
    ftot = 2 * sum(fs)
    units = _units(s_sizes)
    smr_per = H + T * (H + 2)
    smr_per += (-smr_per) % 8
    smr_ones = n_slots * smr_per
    smr_tot = smr_ones + PART

    key = (tuple(s_sizes), D, H, VARIANT)
    if key not in _cache:
        _cache[key] = _build_nc(s_sizes, D, H, VARIANT)
    nc = _cache[key]

    w1f = kd * H
    in_maps = []
    scatter = []  # (core, unit_idx, idx_global_rows)
    for c in range(N_CORES):
        buf = np.zeros((PART, ftot), dtype=np.float32)
        smr = np.zeros((1, smr_tot), dtype=np.float32)
        smr[0, smr_ones:smr_ones + PART] = 1.0
        in_off = 0
        for j, s in enumerate(s_sizes):
            r = j * N_CORES + c
            a = int(rank[r]) if r < A else -1
            if a >= 0:
                n_a = int(counts[a])
                idx = order[starts[a]:starts[a] + n_a]
                kh = kd // 2
                w1p = (W1_embs[a].reshape(kd, PART, H)
                       .transpose(1, 0, 2).reshape(PART, kd * H))
                if n_a > 0:
                    pidx = np.concatenate([idx, np.repeat(idx[:1], s - n_a)])
                    xtp = (X[pidx].T.reshape(kd, PART, s)
                           .transpose(1, 0, 2).reshape(PART, kd * s))
                else:
                    xtp = np.zeros((PART, kd * s), dtype=np.float32)
                for g in range(2):
                    base = in_off + g * fs[j]
                    buf[:, base:base + kh * H] = (
                        w1p[:, g * kh * H:(g + 1) * kh * H])
                    buf[:, base + offs[j]:base + offs[j] + kh * s] = (
                        xtp[:, g * kh * s:(g + 1) * kh * s])
                smr[0, j * smr_per:j * smr_per + H] = b1_embs[a]
                w2c = W2_embs[a].reshape(H, T)
                for t in range(T):
                    base = j * smr_per + H + t * (H + 2)
                    smr[0, base:base + H] = w2c[:, t]
                    smr[0, base + H] = b2_embs[a][t]
                for ui, (jj, s0, sc) in enumerate(units):
                    if jj == j and s0 < n_a:
                        scatter.append((c, ui, idx[s0:s0 + sc]))
            in_off += 2 * fs[j]
        in_maps.append({"IN": buf, "SMALLR": smr})

    if trace:
        _install_ntff_hook()
    from concourse import bass_utils
    bass_utils.upload_artifacts = lambda tmpdir: str(tmpdir)
    res = bass_utils.run_bass_kernel_spmd(
        nc, in_maps, list(range(N_CORES)), trace=trace)

    out = np.zeros((B, T), dtype=np.float32)
    for c, ui, idx in scatter:
        out[idx] = res.results[c]["OUT"][:len(idx), T * ui:T * ui + T]
    return out, res


def kernel(**inputs):
    out, _ = _run(**inputs)
    return out


# revision 22
# speedup vs baseline: 1.4730x; 1.0453x over previous
"""Trainium2 Bass kernel for per-aspect 2-layer MLP (embedding-lookup MLP).

Reference computation (B=1024, D=768, H=256, A=20, T=2):
    W1 = W1_embs[aspect_ids].reshape(B, D, H)
    out1 = relu(X @batched W1 + b1_embs[aspect_ids])
    logits = out1 @batched W2_embs[aspect_ids].reshape(B, H, T) + b2

Strategy: only A=20 distinct aspects exist, so group samples by aspect on
the host and turn the per-sample batched matvec into one dense matmul per
aspect.  Shard the 20 aspect-groups across the 8 NeuronCores (3 slots per
core, groups assigned by size rank so slot j has the same padded size S_j
on every core -> SPMD-uniform program).  Each aspect's W1 (768x256,
786KB) is then read from HBM exactly once across the chip (~16MB total,
~2.4MB per core) instead of once per sample (~800MB).

Device program per slot (S = padded group size, <=128 per chunk):
  - one ~1MB DMA loads the host-packed [128, F] slab: W1 as six [128,256]
    rhs chunks (k-major) + X^T as six [128,S] lhsT chunks.
  - layer 1 on PE: psum[S,256] accumulates 6 matmuls (stationary = X^T
    chunk, moving = W1 chunk, fp32r -> full 1 cycle/row stream rate at
    N=256) + a 7th K=1 matmul (ones[1,S] x b1[1,256]) adding the bias.
  - ScalarE Relu copies psum -> sbuf out1[S,256].
  - layer 2 on DVE: for t in {0,1}, affine_mul_reduce computes
    out1 * w2_t (w2 column DMA-replicated across partitions) with
    accum_out = per-sample sum = logits column; then one tensor_add
    applies b2 (also DMA-replicated).
  - logits live as [S, 2] columns of a persistent [128, 2*n_units] tile;
    one final DMA stores it.

fp32r note: float32r is the TRN2 single-pass fp32 matmul mode (~1.5e-4
relative error vs ~1e-7 for the 2-pass fp32 mode, ~2.7x faster).  Set
VARIANT="fp32" for bit-accurate 2-pass fp32 matmuls.
"""

import numpy as np

N_CORES = 8
PART = 128
VARIANT = "fp32r"  # "fp32r" | "fp32"

_cache: dict = {}


# ───────────────────────── BIR post-pass ─────────────────────────

def _split_excess_waits(nc):
    """This walrus build rejects >1 sync-wait on one instruction (seen on
    the TileContext tail Drain).  Hoist excess sem waits onto preceding
    NoOps on the same engine — semantically identical (program order)."""
    import concourse.mybir as mybir
    import bass_rust

    n_new = 0
    for f in nc.m.functions:
        for bb in f.blocks:
            insts = bb.instructions
            out = []
            changed = False
            for inst in insts:
                si = inst.sync_info
                if si is not None and si.on_wait and len(si.on_wait) > 1:
                    waits = list(si.on_wait)
                    keep = [w for w in waits if w.wait_reg is not None]
                    movable = [w for w in waits if w.wait_reg is None]
                    while len(keep) < 1 and movable:
                        keep.append(movable.pop())
                    for w in movable:
                        nop = mybir.InstNoOp(
                            name=f"waitsplit_{n_new}", engine=inst.engine,
                            sync_info=bass_rust.SyncInfo(on_wait=[w], on_update=[]))
                        n_new += 1
                        out.append(nop)
                    inst.sync_info = bass_rust.SyncInfo(
                        on_wait=keep, on_update=list(si.on_update))
                    changed = True
                out.append(inst)
            if changed:
                bb.instructions = out
    return n_new


def _hoist_initial_dmas(nc):
    """Move wait-free input-DMA triggers from the tile body to before the
    program's entry barrier on their issuing engine, so HBM transfers start
    while the engines are still initializing (saves ~6us of startup)."""
    import concourse.mybir as mybir

    f = nc.m.functions[0]
    bbs = list(f.blocks)
    if len(bbs) < 2:
        return 0
    main_bb, body_bb = bbs[0], bbs[1]

    body = body_bb.instructions
    hoisted = {}  # engine -> list[inst]
    remaining = []
    blocked = set()  # engines whose stream hit a non-hoistable inst
    for inst in body:
        eng = inst.engine
        si = inst.sync_info
        is_dma = isinstance(inst, mybir.InstDMACopy)
        waitfree = si is None or not si.on_wait
        if is_dma and waitfree and eng not in blocked:
            hoisted.setdefault(eng, []).append(inst)
        else:
            if eng != mybir.EngineType.Unassigned:
                blocked.add(eng)
            remaining.append(inst)
    if not hoisted:
        return 0

    main = main_bb.instructions
    out = []
    placed = set()
    # insert after the engine's last InstRegisterMove (before its Drain)
    for i, inst in enumerate(main):
        nxt_is_drain = isinstance(inst, mybir.InstDrain)
        if (inst.engine in hoisted and inst.engine not in placed
                and nxt_is_drain):
            out.extend(hoisted[inst.engine])
            placed.add(inst.engine)
        out.append(inst)
    for eng, insts in hoisted.items():
        if eng not in placed:
            out.extend(insts)
    main_bb.instructions = out
    body_bb.instructions = remaining
    return sum(len(v) for v in hoisted.values())


# ───────────────────────── device program ─────────────────────────

def _layout(s_sizes, d, h):
    """Half-slab layout: per slot TWO slabs (k-groups), each
    [128, (kd/2)*h + (kd/2)*S]: W1 chunks then XT chunks."""
    kd = d // PART
    kh = kd // 2
    offs, fs = [], []
    for s in s_sizes:
        o_xt = kh * h
        f = o_xt + kh * s
        f += (-f) % 8
        offs.append(o_xt)
        fs.append(f)
    return offs, fs


def _units(s_sizes):
    """(slot, s0, sc) chunks of <=128 samples."""
    us = []
    for j, s in enumerate(s_sizes):
        for s0 in range(0, s, PART):
            us.append((j, s0, min(PART, s - s0)))
    return us


def _build_nc(s_sizes, d, h, variant):
    import concourse.bass as bass
    import concourse.mybir as mybir
    from concourse.tile import TileContext

    fp32 = mybir.dt.float32
    # matmul-operand dtype: float32r = single-pass fp32 PE mode
    mmdt = mybir.dt.float32r if variant == "fp32r" else fp32
    kd, mh = d // PART, h // PART
    offs, fs = _layout(s_sizes, d, h)
    ftot = 2 * sum(fs)
    units = _units(s_sizes)
    n_slots = len(s_sizes)

    # SMALLR row layout: per-slot [b1[h] | w2col0,b2_0 [h+1] | w2col1,b2_1
    # [h+1]], then ones[PART]
    T = 2
    smr_per = h + T * (h + 2)
    smr_per += (-smr_per) % 8
    smr_ones = n_slots * smr_per
    smr_tot = smr_ones + PART

    nc = bass.Bass()
    IN = nc.dram_tensor("IN", [PART, ftot], mmdt, kind="ExternalInput")
    SMALLR = nc.dram_tensor("SMALLR", [1, smr_tot], mmdt, kind="ExternalInput")
    OUT = nc.dram_tensor("OUT", [PART, T * len(units)], fp32,
                         kind="ExternalOutput")

    with TileContext(nc) as tc:
        with tc.tile_pool(name="inp", bufs=6) as inp_pool, \
             tc.tile_pool(name="smallp", bufs=1) as small_pool, \
             tc.tile_pool(name="o1p", bufs=2) as o1_pool, \
             tc.tile_pool(name="scrp", bufs=2) as scr_pool, \
             tc.tile_pool(name="outp", bufs=1) as out_pool, \
             tc.tile_pool(name="ps1", bufs=2, space="PSUM") as ps1_pool, \
             tc.tile_pool(name="psw", bufs=T * n_slots, space="PSUM") as psw_pool:

            out_sb = out_pool.tile([PART, T * len(units)], fp32)
            small_t = small_pool.tile([1, smr_tot], mmdt)
            nc.scalar.dma_start(out=small_t[:], in_=SMALLR[:])

            # prefetch all input half-slabs on ONE ring in consumption
            # order (per-ring FIFO => data lands exactly in compute order;
            # a second ring would only steal bandwidth from the head).
            in_ts = []
            in_off = 0
            for j, s in enumerate(s_sizes):
                pair = []
                for g in range(2):
                    in_t = inp_pool.tile([PART, fs[j]], mmdt, tag="in_t")
                    eng = nc.sync if g == 0 else nc.scalar
                    eng.dma_start(
                        out=in_t[:], in_=IN[:, in_off:in_off + fs[j]])
                    pair.append(in_t)
                    in_off += fs[j]
                in_ts.append(pair)

            ones_full = small_t[0:1, smr_ones:smr_ones + PART]

            # replicate each slot's w2 column (+b2) across all partitions on
            # the PE: psum[p, f] = ones[p] * w2row[f]; DVE then reads PSUM.
            w2ps = []
            for j in range(n_slots):
                for t in range(T):
                    wp = psw_pool.tile([PART, h + 2], fp32, tag="w2ps")
                    src = small_t[0:1, j * smr_per + h + t * (h + 2):
                                  j * smr_per + h + (t + 1) * (h + 2)]
                    nc.tensor.matmul(wp[:], ones_full, src,
                                     start=True, stop=True)
                    w2ps.append(wp)

            for ui, (j, s0, sc) in enumerate(units):
                s = s_sizes[j]
                o_xt = offs[j]
                kh = kd // 2
                b1row = small_t[0:1, j * smr_per:j * smr_per + h]
                ones = small_t[0:1, smr_ones + 0:smr_ones + sc]

                ps = ps1_pool.tile([sc, h], fp32, tag="ps")
                for k in range(kd):
                    in_t = in_ts[j][k // kh]
                    kk = k % kh
                    nc.tensor.matmul(
                        ps[:],
                        in_t[:, o_xt + kk * s + s0:o_xt + kk * s + s0 + sc],
                        in_t[:, kk * h:(kk + 1) * h],
                        start=(k == 0), stop=False)
                nc.tensor.matmul(
                    ps[:], ones, b1row, start=False, stop=True)

                o1 = o1_pool.tile([PART, h], fp32, tag="o1")
                nc.scalar.activation(
                    o1[:sc], ps[:], mybir.ActivationFunctionType.Relu)

                scr = scr_pool.tile([PART, T * h], fp32, tag="scr")
                acc = scr_pool.tile([PART, T], fp32, tag="acc")
                for t in range(T):
                    wp = w2ps[j * T + t]
                    nc.vector.tensor_mul(
                        out=scr[:sc, t * h:(t + 1) * h], in0=o1[:sc],
                        in1=wp[:sc, 0:h])
                    nc.vector.reduce_sum(
                        out=acc[:sc, t:t + 1], in_=scr[:sc, t * h:(t + 1) * h],
                        axis=mybir.AxisListType.X)
                    nc.vector.tensor_add(
                        out=out_sb[:sc, T * ui + t:T * ui + t + 1],
                        in0=acc[:sc, t:t + 1],
                        in1=wp[:sc, h:h + 1])
            nc.scalar.dma_start(out=OUT[:], in_=out_sb[:])

    _split_excess_waits(nc)
    _hoist_initial_dmas(nc)
    return nc


# ───────────────────────── host side ─────────────────────────

def _install_ntff_hook():
    import sys, types
    if "antenv.axon_hooks" in sys.modules:
        return
    import antenv
    from trn_agent_boot.trn_boot import _ntff_profile_via_ctypes
    mod = types.ModuleType("antenv.axon_hooks")
    hook = _ntff_profile_via_ctypes('/opt/axon/libaxon_pjrt.so')
    mod.get_axon_ntff_profile_hook = lambda: hook
    mod.set_axon_ntff_profile_hook = lambda h: None
    sys.modules["antenv.axon_hooks"] = mod
    antenv.axon_hooks = mod


def _run(X, aspect_ids, W1_embs, b1_embs, W2_embs, b2_embs, trace=False):
    B, D = X.shape
    A, H = b1_embs.shape
    T = b2_embs.shape[1]
    assert D % PART == 0 and H % PART == 0 and T == 2
    kd, mh = D // PART, H // PART

    X = np.ascontiguousarray(X, dtype=np.float32)
    W1_embs = np.ascontiguousarray(W1_embs, dtype=np.float32)
    b1_embs = np.ascontiguousarray(b1_embs, dtype=np.float32)
    W2_embs = np.ascontiguousarray(W2_embs, dtype=np.float32)
    b2_embs = np.ascontiguousarray(b2_embs, dtype=np.float32)
    ids = np.asarray(aspect_ids).astype(np.int64)

    order = np.argsort(ids, kind="stable")
    counts = np.bincount(ids, minlength=A)
    starts = np.concatenate([[0], np.cumsum(counts)])
    rank = np.argsort(-counts, kind="stable")

    n_slots = -(-A // N_CORES)
    s_sizes = []
    for j in range(n_slots):
        cls = rank[j * N_CORES:(j + 1) * N_CORES]
        smax = max(1, int(counts[cls].max()))
        smax += (-smax) % 8
        s_sizes.append(smax)

    offs, fs = _layout(s_sizes, D, H)
    ftot = 2 * sum(fs)
    units = _units(s_sizes)
    smr_per = H + T * (H + 2)
    smr_per += (-smr_per) % 8
    smr_ones = n_slots * smr_per
    smr_tot = smr_ones + PART

    key = (tuple(s_sizes), D, H, VARIANT)
    if key not in _cache:
        _cache[key] = _build_nc(s_sizes, D, H, VARIANT)
    nc = _cache[key]

    w1f = kd * H
    in_maps = []
    scatter = []  # (core, unit_idx, idx_global_rows)
    for c in range(N_CORES):
        buf = np.zeros((PART, ftot), dtype=np.float32)
        smr = np.zeros((1, smr_tot), dtype=np.float32)
        smr[0, smr_ones:smr_ones + PART] = 1.0
        in_off = 0
        for j, s in enumerate(s_sizes):
            r = j * N_CORES + c
            a = int(rank[r]) if r < A else -1
            if a >= 0:
                n_a = int(counts[a])
                idx = order[starts[a]:starts[a] + n_a]
                kh = kd // 2
                w1p = (W1_embs[a].reshape(kd, PART, H)
                       .transpose(1, 0, 2).reshape(PART, kd * H))
                if n_a > 0:
                    pidx = np.concatenate([idx, np.repeat(idx[:1], s - n_a)])
                    xtp = (X[pidx].T.reshape(kd, PART, s)
                           .transpose(1, 0, 2).reshape(PART, kd * s))
                else:
                    xtp = np.zeros((PART, kd * s), dtype=np.float32)
                for g in range(2):
                    base = in_off + g * fs[j]
                    buf[:, base:base + kh * H] = (
                        w1p[:, g * kh * H:(g + 1) * kh * H])
                    buf[:, base + offs[j]:base + offs[j] + kh * s] = (
                        xtp[:, g * kh * s:(g + 1) * kh * s])
                smr[0, j * smr_per:j * smr_per + H] = b1_embs[a]
                w2c = W2_embs[a].reshape(H, T)
                for t in range(T):
                    base = j * smr_per + H + t * (H + 2)
                    smr[0, base:base + H] = w2c[:, t]
                    smr[0, base + H] = b2_embs[a][t]
                for ui, (jj, s0, sc) in enumerate(units):
                    if jj == j and s0 < n_a:
                        scatter.append((c, ui, idx[s0:s0 + sc]))
            in_off += 2 * fs[j]
        in_maps.append({"IN": buf, "SMALLR": smr})

    if trace:
        _install_ntff_hook()
    from concourse import bass_utils
    bass_utils.upload_artifacts = lambda tmpdir: str(tmpdir)
    res = bass_utils.run_bass_kernel_spmd(
        nc, in_maps, list(range(N_CORES)), trace=trace)

    out = np.zeros((B, T), dtype=np.float32)
    for c, ui, idx in scatter:
        out[idx] = res.results[c]["OUT"][:len(idx), T * ui:T * ui + T]
    return out, res


def kernel(**inputs):
    out, _ = _run(**inputs)
    return out


# revision 23
# speedup vs baseline: 1.5664x; 1.0635x over previous
"""Trainium2 Bass kernel for per-aspect 2-layer MLP (embedding-lookup MLP).

Reference computation (B=1024, D=768, H=256, A=20, T=2):
    W1 = W1_embs[aspect_ids].reshape(B, D, H)
    out1 = relu(X @batched W1 + b1_embs[aspect_ids])
    logits = out1 @batched W2_embs[aspect_ids].reshape(B, H, T) + b2

Strategy: only A=20 distinct aspects exist, so group samples by aspect on
the host and turn the per-sample batched matvec into one dense matmul per
aspect.  Shard the 20 aspect-groups across the 8 NeuronCores (3 slots per
core, groups assigned by size rank so slot j has the same padded size S_j
on every core -> SPMD-uniform program).  Each aspect's W1 (768x256,
786KB) is then read from HBM exactly once across the chip (~16MB total,
~2.4MB per core) instead of once per sample (~800MB).

Device program per slot (S = padded group size, <=128 per chunk):
  - one ~1MB DMA loads the host-packed [128, F] slab: W1 as six [128,256]
    rhs chunks (k-major) + X^T as six [128,S] lhsT chunks.
  - layer 1 on PE: psum[S,256] accumulates 6 matmuls (stationary = X^T
    chunk, moving = W1 chunk, fp32r -> full 1 cycle/row stream rate at
    N=256) + a 7th K=1 matmul (ones[1,S] x b1[1,256]) adding the bias.
  - ScalarE Relu copies psum -> sbuf out1[S,256].
  - layer 2 on DVE: for t in {0,1}, affine_mul_reduce computes
    out1 * w2_t (w2 column DMA-replicated across partitions) with
    accum_out = per-sample sum = logits column; then one tensor_add
    applies b2 (also DMA-replicated).
  - logits live as [S, 2] columns of a persistent [128, 2*n_units] tile;
    one final DMA stores it.

fp32r note: float32r is the TRN2 single-pass fp32 matmul mode (~1.5e-4
relative error vs ~1e-7 for the 2-pass fp32 mode, ~2.7x faster).  Set
VARIANT="fp32" for bit-accurate 2-pass fp32 matmuls.
"""

import numpy as np

N_CORES = 8
PART = 128
VARIANT = "fp32r"  # "fp32r" | "fp32"

_cache: dict = {}


# ───────────────────────── BIR post-pass ─────────────────────────

def _split_excess_waits(nc):
    """This walrus build rejects >1 sync-wait on one instruction (seen on
    the TileContext tail Drain).  Hoist excess sem waits onto preceding
    NoOps on the same engine — semantically identical (program order)."""
    import concourse.mybir as mybir
    import bass_rust

    n_new = 0
    for f in nc.m.functions:
        for bb in f.blocks:
            insts = bb.instructions
            out = []
            changed = False
            for inst in insts:
                si = inst.sync_info
                if si is not None and si.on_wait and len(si.on_wait) > 1:
                    waits = list(si.on_wait)
                    keep = [w for w in waits if w.wait_reg is not None]
                    movable = [w for w in waits if w.wait_reg is None]
                    while len(keep) < 1 and movable:
                        keep.append(movable.pop())
                    for w in movable:
                        nop = mybir.InstNoOp(
                            name=f"waitsplit_{n_new}", engine=inst.engine,
                            sync_info=bass_rust.SyncInfo(on_wait=[w], on_update=[]))
                        n_new += 1
                        out.append(nop)
                    inst.sync_info = bass_rust.SyncInfo(
                        on_wait=keep, on_update=list(si.on_update))
                    changed = True
                out.append(inst)
            if changed:
                bb.instructions = out
    return n_new


def _hoist_initial_dmas(nc):
    """Move wait-free input-DMA triggers from the tile body to before the
    program's entry barrier on their issuing engine, so HBM transfers start
    while the engines are still initializing (saves ~6us of startup)."""
    import concourse.mybir as mybir

    f = nc.m.functions[0]
    bbs = list(f.blocks)
    if len(bbs) < 2:
        return 0
    main_bb, body_bb = bbs[0], bbs[1]

    body = body_bb.instructions
    hoisted = {}  # engine -> list[inst]
    remaining = []
    blocked = set()  # engines whose stream hit a non-hoistable inst
    for inst in body:
        eng = inst.engine
        si = inst.sync_info
        is_dma = isinstance(inst, mybir.InstDMACopy)
        waitfree = si is None or not si.on_wait
        if is_dma and waitfree and eng not in blocked:
            hoisted.setdefault(eng, []).append(inst)
        else:
            if eng != mybir.EngineType.Unassigned:
                blocked.add(eng)
            remaining.append(inst)
    if not hoisted:
        return 0

    main = main_bb.instructions
    out = []
    placed = set()
    # insert after the engine's last InstRegisterMove (before its Drain)
    for i, inst in enumerate(main):
        nxt_is_drain = isinstance(inst, mybir.InstDrain)
        if (inst.engine in hoisted and inst.engine not in placed
                and nxt_is_drain):
            out.extend(hoisted[inst.engine])
            placed.add(inst.engine)
        out.append(inst)
    for eng, insts in hoisted.items():
        if eng not in placed:
            out.extend(insts)
    main_bb.instructions = out
    body_bb.instructions = remaining
    return sum(len(v) for v in hoisted.values())


# ───────────────────────── device program ─────────────────────────

def _layout(s_sizes, d, h):
    """Half-slab layout: per slot TWO slabs (k-groups), each
    [128, (kd/2)*h + (kd/2)*S]: W1 chunks then XT chunks."""
    kd = d // PART
    kh = kd // 2
    offs, fs = [], []
    for s in s_sizes:
        o_xt = kh * h
        f = o_xt + kh * s
        f += (-f) % 8
        offs.append(o_xt)
        fs.append(f)
    return offs, fs


def _units(s_sizes):
    """(slot, s0, sc) chunks of <=128 samples."""
    us = []
    for j, s in enumerate(s_sizes):
        for s0 in range(0, s, PART):
            us.append((j, s0, min(PART, s - s0)))
    return us


def _build_nc(s_sizes, d, h, variant):
    import concourse.bass as bass
    import concourse.mybir as mybir
    from concourse.tile import TileContext

    fp32 = mybir.dt.float32
    # matmul-operand dtype: float32r = single-pass fp32 PE mode
    mmdt = mybir.dt.float32r if variant == "fp32r" else fp32
    kd, mh = d // PART, h // PART
    offs, fs = _layout(s_sizes, d, h)
    ftot = 2 * sum(fs)
    units = _units(s_sizes)
    n_slots = len(s_sizes)

    # SMALLR row layout: per-slot [b1[h] | w2col0,b2_0 [h+1] | w2col1,b2_1
    # [h+1]], then ones[PART]
    T = 2
    smr_per = h + T * (h + 2)
    smr_per += (-smr_per) % 8
    smr_ones = n_slots * smr_per
    smr_tot = smr_ones + PART

    nc = bass.Bass()
    IN = nc.dram_tensor("IN", [PART, ftot], mmdt, kind="ExternalInput")
    SMALLR = nc.dram_tensor("SMALLR", [1, smr_tot], mmdt, kind="ExternalInput")
    OUT = nc.dram_tensor("OUT", [PART, T * len(units)], fp32,
                         kind="ExternalOutput")

    with TileContext(nc) as tc:
        with tc.tile_pool(name="inp", bufs=6) as inp_pool, \
             tc.tile_pool(name="smallp", bufs=1) as small_pool, \
             tc.tile_pool(name="w2sbp", bufs=6) as w2sb_pool, \
             tc.tile_pool(name="scrp", bufs=2) as scr_pool, \
             tc.tile_pool(name="outp", bufs=1) as out_pool, \
             tc.tile_pool(name="ps1", bufs=2, space="PSUM") as ps1_pool, \
             tc.tile_pool(name="psw", bufs=T * n_slots, space="PSUM") as psw_pool:

            out_sb = out_pool.tile([PART, T * len(units)], fp32)
            small_t = small_pool.tile([1, smr_tot], mmdt)
            nc.scalar.dma_start(out=small_t[:], in_=SMALLR[:])

            # prefetch all input half-slabs on ONE ring in consumption
            # order (per-ring FIFO => data lands exactly in compute order;
            # a second ring would only steal bandwidth from the head).
            in_ts = []
            in_off = 0
            for j, s in enumerate(s_sizes):
                pair = []
                for g in range(2):
                    in_t = inp_pool.tile([PART, fs[j]], mmdt, tag="in_t")
                    eng = nc.sync if g == 0 else nc.scalar
                    eng.dma_start(
                        out=in_t[:], in_=IN[:, in_off:in_off + fs[j]])
                    pair.append(in_t)
                    in_off += fs[j]
                in_ts.append(pair)

            ones_full = small_t[0:1, smr_ones:smr_ones + PART]

            # replicate each slot's w2 column (+b2) across all partitions on
            # the PE: psum[p, f] = ones[p] * w2row[f]; copy to SBUF for DVE.
            w2sb = []
            for j in range(n_slots):
                for t in range(T):
                    wp = psw_pool.tile([PART, h + 2], fp32, tag="w2ps")
                    src = small_t[0:1, j * smr_per + h + t * (h + 2):
                                  j * smr_per + h + (t + 1) * (h + 2)]
                    nc.tensor.matmul(wp[:], ones_full, src,
                                     start=True, stop=True)
                    wsb = w2sb_pool.tile([PART, h + 2], fp32, tag="w2sb")
                    nc.vector.tensor_copy(out=wsb[:], in_=wp[:])
                    w2sb.append(wsb)

            for ui, (j, s0, sc) in enumerate(units):
                s = s_sizes[j]
                o_xt = offs[j]
                kh = kd // 2
                b1row = small_t[0:1, j * smr_per:j * smr_per + h]
                ones = small_t[0:1, smr_ones + 0:smr_ones + sc]

                ps = ps1_pool.tile([sc, h], fp32, tag="ps")
                for k in range(kd):
                    in_t = in_ts[j][k // kh]
                    kk = k % kh
                    nc.tensor.matmul(
                        ps[:],
                        in_t[:, o_xt + kk * s + s0:o_xt + kk * s + s0 + sc],
                        in_t[:, kk * h:(kk + 1) * h],
                        start=(k == 0), stop=False)
                nc.tensor.matmul(
                    ps[:], ones, b1row, start=False, stop=True)

                scr = scr_pool.tile([PART, T * h], fp32, tag="scr")
                acc = scr_pool.tile([PART, T], fp32, tag="acc")
                for t in range(T):
                    wsb = w2sb[j * T + t]
                    # logits col = sum_h(relu(psum) * w2[:,t]) in one DVE op
                    nc.vector.scalar_tensor_tensor(
                        out=scr[:sc, t * h:(t + 1) * h],
                        in0=ps[:], scalar=0.0, in1=wsb[:sc, 0:h],
                        op0=mybir.AluOpType.max,
                        op1=mybir.AluOpType.mult,
                        accum_out=acc[:sc, t:t + 1])
                    nc.vector.tensor_add(
                        out=out_sb[:sc, T * ui + t:T * ui + t + 1],
                        in0=acc[:sc, t:t + 1],
                        in1=wsb[:sc, h:h + 1])
            nc.scalar.dma_start(out=OUT[:], in_=out_sb[:])

    _split_excess_waits(nc)
    _hoist_initial_dmas(nc)
    return nc


# ───────────────────────── host side ─────────────────────────

def _install_ntff_hook():
    import sys, types
    if "antenv.axon_hooks" in sys.modules:
        return
    import antenv
    from trn_agent_boot.trn_boot import _ntff_profile_via_ctypes
    mod = types.ModuleType("antenv.axon_hooks")
    hook = _ntff_profile_via_ctypes('/opt/axon/libaxon_pjrt.so')
    mod.get_axon_ntff_profile_hook = lambda: hook
    mod.set_axon_ntff_profile_hook = lambda h: None
    sys.modules["antenv.axon_hooks"] = mod
    antenv.axon_hooks = mod


def _run(X, aspect_ids, W1_embs, b1_embs, W2_embs, b2_embs, trace=False):
    B, D = X.shape
    A, H = b1_embs.shape
    T = b2_embs.shape[1]
    assert D % PART == 0 and H % PART == 0 and T == 2
    kd, mh = D // PART, H // PART

    X = np.ascontiguousarray(X, dtype=np.float32)
    W1_embs = np.ascontiguousarray(W1_embs, dtype=np.float32)
    b1_embs = np.ascontiguousarray(b1_embs, dtype=np.float32)
    W2_embs = np.ascontiguousarray(W2_embs, dtype=np.float32)
    b2_embs = np.ascontiguousarray(b2_embs, dtype=np.float32)
    ids = np.asarray(aspect_ids).astype(np.int64)

    order = np.argsort(ids, kind="stable")
    counts = np.bincount(ids, minlength=A)
    starts = np.concatenate([[0], np.cumsum(counts)])
    rank = np.argsort(-counts, kind="stable")

    n_slots = -(-A // N_CORES)
    s_sizes = []
    for j in range(n_slots):
        cls = rank[j * N_CORES:(j + 1) * N_CORES]
        smax = max(1, int(counts[cls].max()))
        smax += (-smax) % 8
        s_sizes.append(smax)

    offs, fs = _layout(s_sizes, D, H)
    ftot = 2 * sum(fs)
    units = _units(s_sizes)
    smr_per = H + T * (H + 2)
    smr_per += (-smr_per) % 8
    smr_ones = n_slots * smr_per
    smr_tot = smr_ones + PART

    key = (tuple(s_sizes), D, H, VARIANT)
    if key not in _cache:
        _cache[key] = _build_nc(s_sizes, D, H, VARIANT)
    nc = _cache[key]

    w1f = kd * H
    in_maps = []
    scatter = []  # (core, unit_idx, idx_global_rows)
    for c in range(N_CORES):
        buf = np.zeros((PART, ftot), dtype=np.float32)
        smr = np.zeros((1, smr_tot), dtype=np.float32)
        smr[0, smr_ones:smr_ones + PART] = 1.0
        in_off = 0
        for j, s in enumerate(s_sizes):
            r = j * N_CORES + c
            a = int(rank[r]) if r < A else -1
            if a >= 0:
                n_a = int(counts[a])
                idx = order[starts[a]:starts[a] + n_a]
                kh = kd // 2
                w1p = (W1_embs[a].reshape(kd, PART, H)
                       .transpose(1, 0, 2).reshape(PART, kd * H))
                if n_a > 0:
                    pidx = np.concatenate([idx, np.repeat(idx[:1], s - n_a)])
                    xtp = (X[pidx].T.reshape(kd, PART, s)
                           .transpose(1, 0, 2).reshape(PART, kd * s))
                else:
                    xtp = np.zeros((PART, kd * s), dtype=np.float32)
                for g in range(2):
                    base = in_off + g * fs[j]
                    buf[:, base:base + kh * H] = (
                        w1p[:, g * kh * H:(g + 1) * kh * H])
                    buf[:, base + offs[j]:base + offs[j] + kh * s] = (
                        xtp[:, g * kh * s:(g + 1) * kh * s])
                smr[0, j * smr_per:j * smr_per + H] = b1_embs[a]
                w2c = W2_embs[a].reshape(H, T)
                for t in range(T):
                    base = j * smr_per + H + t * (H + 2)
                    smr[0, base:base + H] = w2c[:, t]
                    smr[0, base + H] = b2_embs[a][t]
                for ui, (jj, s0, sc) in enumerate(units):
                    if jj == j and s0 < n_a:
                        scatter.append((c, ui, idx[s0:s0 + sc]))
            in_off += 2 * fs[j]
        in_maps.append({"IN": buf, "SMALLR": smr})

    if trace:
        _install_ntff_hook()
    from concourse import bass_utils
    bass_utils.upload_artifacts = lambda tmpdir: str(tmpdir)
    res = bass_utils.run_bass_kernel_spmd(
        nc, in_maps, list(range(N_CORES)), trace=trace)

    out = np.zeros((B, T), dtype=np.float32)
    for c, ui, idx in scatter:
        out[idx] = res.results[c]["OUT"][:len(idx), T * ui:T * ui + T]
    return out, res


def kernel(**inputs):
    out, _ = _run(**inputs)
    return out


# revision 24
# speedup vs baseline: 1.5667x; 1.0002x over previous
"""Trainium2 Bass kernel for per-aspect 2-layer MLP (embedding-lookup MLP).

Reference computation (B=1024, D=768, H=256, A=20, T=2):
    W1 = W1_embs[aspect_ids].reshape(B, D, H)
    out1 = relu(X @batched W1 + b1_embs[aspect_ids])
    logits = out1 @batched W2_embs[aspect_ids].reshape(B, H, T) + b2

Strategy: only A=20 distinct aspects exist, so group samples by aspect on
the host and turn the per-sample batched matvec into one dense matmul per
aspect.  Shard the 20 aspect-groups across the 8 NeuronCores (3 slots per
core, groups assigned by size rank so slot j has the same padded size S_j
on every core -> SPMD-uniform program).  Each aspect's W1 (768x256,
786KB) is then read from HBM exactly once across the chip (~16MB total,
~2.4MB per core) instead of once per sample (~800MB).

Device program per slot (S = padded group size, <=128 per chunk):
  - one ~1MB DMA loads the host-packed [128, F] slab: W1 as six [128,256]
    rhs chunks (k-major) + X^T as six [128,S] lhsT chunks.
  - layer 1 on PE: psum[S,256] accumulates 6 matmuls (stationary = X^T
    chunk, moving = W1 chunk, fp32r -> full 1 cycle/row stream rate at
    N=256) + a 7th K=1 matmul (ones[1,S] x b1[1,256]) adding the bias.
  - ScalarE Relu copies psum -> sbuf out1[S,256].
  - layer 2 on DVE: for t in {0,1}, affine_mul_reduce computes
    out1 * w2_t (w2 column DMA-replicated across partitions) with
    accum_out = per-sample sum = logits column; then one tensor_add
    applies b2 (also DMA-replicated).
  - logits live as [S, 2] columns of a persistent [128, 2*n_units] tile;
    one final DMA stores it.

fp32r note: float32r is the TRN2 single-pass fp32 matmul mode (~1.5e-4
relative error vs ~1e-7 for the 2-pass fp32 mode, ~2.7x faster).  Set
VARIANT="fp32" for bit-accurate 2-pass fp32 matmuls.
"""

import numpy as np

N_CORES = 8
PART = 128
VARIANT = "fp32r"  # "fp32r" | "fp32"

_cache: dict = {}


# ───────────────────────── BIR post-pass ─────────────────────────

def _split_excess_waits(nc):
    """This walrus build rejects >1 sync-wait on one instruction (seen on
    the TileContext tail Drain).  Hoist excess sem waits onto preceding
    NoOps on the same engine — semantically identical (program order)."""
    import concourse.mybir as mybir
    import bass_rust

    n_new = 0
    for f in nc.m.functions:
        for bb in f.blocks:
            insts = bb.instructions
            out = []
            changed = False
            for inst in insts:
                si = inst.sync_info
                if si is not None and si.on_wait and len(si.on_wait) > 1:
                    waits = list(si.on_wait)
                    keep = [w for w in waits if w.wait_reg is not None]
                    movable = [w for w in waits if w.wait_reg is None]
                    while len(keep) < 1 and movable:
                        keep.append(movable.pop())
                    for w in movable:
                        nop = mybir.InstNoOp(
                            name=f"waitsplit_{n_new}", engine=inst.engine,
                            sync_info=bass_rust.SyncInfo(on_wait=[w], on_update=[]))
                        n_new += 1
                        out.append(nop)
                    inst.sync_info = bass_rust.SyncInfo(
                        on_wait=keep, on_update=list(si.on_update))
                    changed = True
                out.append(inst)
            if changed:
                bb.instructions = out
    return n_new


def _hoist_initial_dmas(nc):
    """Move wait-free input-DMA triggers from the tile body to before the
    program's entry barrier on their issuing engine, so HBM transfers start
    while the engines are still initializing (saves ~6us of startup)."""
    import concourse.mybir as mybir

    f = nc.m.functions[0]
    bbs = list(f.blocks)
    if len(bbs) < 2:
        return 0
    main_bb, body_bb = bbs[0], bbs[1]

    body = body_bb.instructions
    hoisted = {}  # engine -> list[inst]
    remaining = []
    blocked = set()  # engines whose stream hit a non-hoistable inst
    for inst in body:
        eng = inst.engine
        si = inst.sync_info
        is_dma = isinstance(inst, mybir.InstDMACopy)
        waitfree = si is None or not si.on_wait
        if is_dma and waitfree and eng not in blocked:
            hoisted.setdefault(eng, []).append(inst)
        else:
            if eng != mybir.EngineType.Unassigned:
                blocked.add(eng)
            remaining.append(inst)
    if not hoisted:
        return 0

    main = main_bb.instructions
    out = []
    placed = set()
    # insert after the engine's last InstRegisterMove (before its Drain)
    for i, inst in enumerate(main):
        nxt_is_drain = isinstance(inst, mybir.InstDrain)
        if (inst.engine in hoisted and inst.engine not in placed
                and nxt_is_drain):
            out.extend(hoisted[inst.engine])
            placed.add(inst.engine)
        out.append(inst)
    for eng, insts in hoisted.items():
        if eng not in placed:
            out.extend(insts)
    main_bb.instructions = out
    body_bb.instructions = remaining
    return sum(len(v) for v in hoisted.values())


# ───────────────────────── device program ─────────────────────────

def _layout(s_sizes, d, h):
    """Half-slab layout: per slot TWO slabs (k-groups), each
    [128, (kd/2)*h + (kd/2)*S]: W1 chunks then XT chunks."""
    kd = d // PART
    kh = kd // 2
    offs, fs = [], []
    for s in s_sizes:
        o_xt = kh * h
        f = o_xt + kh * s
        f += (-f) % 8
        offs.append(o_xt)
        fs.append(f)
    return offs, fs


def _units(s_sizes):
    """(slot, s0, sc) chunks of <=128 samples."""
    us = []
    for j, s in enumerate(s_sizes):
        for s0 in range(0, s, PART):
            us.append((j, s0, min(PART, s - s0)))
    return us


def _build_nc(s_sizes, d, h, variant):
    import concourse.bass as bass
    import concourse.mybir as mybir
    from concourse.tile import TileContext

    fp32 = mybir.dt.float32
    # matmul-operand dtype: float32r = single-pass fp32 PE mode
    mmdt = mybir.dt.float32r if variant == "fp32r" else fp32
    kd, mh = d // PART, h // PART
    offs, fs = _layout(s_sizes, d, h)
    ftot = 2 * sum(fs)
    units = _units(s_sizes)
    n_slots = len(s_sizes)

    # SMALLR row layout: per-slot [b1[h] | w2col0,b2_0 [h+1] | w2col1,b2_1
    # [h+1]], then ones[PART]
    T = 2
    smr_per = h + T * (h + 2)
    smr_per += (-smr_per) % 8
    smr_ones = n_slots * smr_per
    smr_tot = smr_ones + PART

    nc = bass.Bass()
    IN = nc.dram_tensor("IN", [PART, ftot], mmdt, kind="ExternalInput")
    SMALLR = nc.dram_tensor("SMALLR", [1, smr_tot], mmdt, kind="ExternalInput")
    OUT = nc.dram_tensor("OUT", [PART, T * len(units)], fp32,
                         kind="ExternalOutput")

    with TileContext(nc) as tc:
        with tc.tile_pool(name="inp", bufs=6) as inp_pool, \
             tc.tile_pool(name="smallp", bufs=1) as small_pool, \
             tc.tile_pool(name="w2sbp", bufs=6) as w2sb_pool, \
             tc.tile_pool(name="scrp", bufs=2) as scr_pool, \
             tc.tile_pool(name="outp", bufs=1) as out_pool, \
             tc.tile_pool(name="ps1", bufs=2, space="PSUM") as ps1_pool, \
             tc.tile_pool(name="psw", bufs=T * n_slots, space="PSUM") as psw_pool:

            out_sb = out_pool.tile([PART, T * len(units)], fp32)
            small_t = small_pool.tile([1, smr_tot], mmdt)

            # prefetch all input half-slabs, k<kd/2 halves on the sync ring
            # and k>=kd/2 halves on the scalar ring (per-ring FIFO => data
            # lands in compute order; two rings keep two transfers in
            # flight, which measures ~25% faster than one).  Slot-0's
            # second half goes FIRST on the scalar ring so slot 0 completes
            # as early as possible.
            in_ts = []
            in_off = 0
            for j, s in enumerate(s_sizes):
                pair = []
                for g in range(2):
                    in_t = inp_pool.tile([PART, fs[j]], mmdt, tag="in_t")
                    eng = nc.sync if g == 0 else nc.scalar
                    eng.dma_start(
                        out=in_t[:], in_=IN[:, in_off:in_off + fs[j]])
                    pair.append(in_t)
                    in_off += fs[j]
                    if j == 0 and g == 1:
                        nc.scalar.dma_start(out=small_t[:], in_=SMALLR[:])
                in_ts.append(pair)

            ones_full = small_t[0:1, smr_ones:smr_ones + PART]

            # replicate each slot's w2 column (+b2) across all partitions on
            # the PE: psum[p, f] = ones[p] * w2row[f]; copy to SBUF for DVE.
            w2sb = []
            for j in range(n_slots):
                for t in range(T):
                    wp = psw_pool.tile([PART, h + 2], fp32, tag="w2ps")
                    src = small_t[0:1, j * smr_per + h + t * (h + 2):
                                  j * smr_per + h + (t + 1) * (h + 2)]
                    nc.tensor.matmul(wp[:], ones_full, src,
                                     start=True, stop=True)
                    wsb = w2sb_pool.tile([PART, h + 2], fp32, tag="w2sb")
                    nc.vector.tensor_copy(out=wsb[:], in_=wp[:])
                    w2sb.append(wsb)

            for ui, (j, s0, sc) in enumerate(units):
                s = s_sizes[j]
                o_xt = offs[j]
                kh = kd // 2
                b1row = small_t[0:1, j * smr_per:j * smr_per + h]
                ones = small_t[0:1, smr_ones + 0:smr_ones + sc]

                ps = ps1_pool.tile([sc, h], fp32, tag="ps")
                for k in range(kd):
                    in_t = in_ts[j][k // kh]
                    kk = k % kh
                    nc.tensor.matmul(
                        ps[:],
                        in_t[:, o_xt + kk * s + s0:o_xt + kk * s + s0 + sc],
                        in_t[:, kk * h:(kk + 1) * h],
                        start=(k == 0), stop=False)
                nc.tensor.matmul(
                    ps[:], ones, b1row, start=False, stop=True)

                scr = scr_pool.tile([PART, T * h], fp32, tag="scr")
                acc = scr_pool.tile([PART, T], fp32, tag="acc")
                for t in range(T):
                    wsb = w2sb[j * T + t]
                    # logits col = sum_h(relu(psum) * w2[:,t]) in one DVE op
                    nc.vector.scalar_tensor_tensor(
                        out=scr[:sc, t * h:(t + 1) * h],
                        in0=ps[:], scalar=0.0, in1=wsb[:sc, 0:h],
                        op0=mybir.AluOpType.max,
                        op1=mybir.AluOpType.mult,
                        accum_out=acc[:sc, t:t + 1])
                    nc.vector.tensor_add(
                        out=out_sb[:sc, T * ui + t:T * ui + t + 1],
                        in0=acc[:sc, t:t + 1],
                        in1=wsb[:sc, h:h + 1])
            nc.scalar.dma_start(out=OUT[:], in_=out_sb[:])

    _split_excess_waits(nc)
    _hoist_initial_dmas(nc)
    return nc


# ───────────────────────── host side ─────────────────────────

def _install_ntff_hook():
    import sys, types
    if "antenv.axon_hooks" in sys.modules:
        return
    import antenv
    from trn_agent_boot.trn_boot import _ntff_profile_via_ctypes
    mod = types.ModuleType("antenv.axon_hooks")
    hook = _ntff_profile_via_ctypes('/opt/axon/libaxon_pjrt.so')
    mod.get_axon_ntff_profile_hook = lambda: hook
    mod.set_axon_ntff_profile_hook = lambda h: None
    sys.modules["antenv.axon_hooks"] = mod
    antenv.axon_hooks = mod


def _run(X, aspect_ids, W1_embs, b1_embs, W2_embs, b2_embs, trace=False):
    B, D = X.shape
    A, H = b1_embs.shape
    T = b2_embs.shape[1]
    assert D % PART == 0 and H % PART == 0 and T == 2
    kd, mh = D // PART, H // PART

    X = np.ascontiguousarray(X, dtype=np.float32)
    W1_embs = np.ascontiguousarray(W1_embs, dtype=np.float32)
    b1_embs = np.ascontiguousarray(b1_embs, dtype=np.float32)
    W2_embs = np.ascontiguousarray(W2_embs, dtype=np.float32)
    b2_embs = np.ascontiguousarray(b2_embs, dtype=np.float32)
    ids = np.asarray(aspect_ids).astype(np.int64)

    order = np.argsort(ids, kind="stable")
    counts = np.bincount(ids, minlength=A)
    starts = np.concatenate([[0], np.cumsum(counts)])
    rank = np.argsort(-counts, kind="stable")

    n_slots = -(-A // N_CORES)
    s_sizes = []
    for j in range(n_slots):
        cls = rank[j * N_CORES:(j + 1) * N_CORES]
        smax = max(1, int(counts[cls].max()))
        smax += (-smax) % 8
        s_sizes.append(smax)

    offs, fs = _layout(s_sizes, D, H)
    ftot = 2 * sum(fs)
    units = _units(s_sizes)
    smr_per = H + T * (H + 2)
    smr_per += (-smr_per) % 8
    smr_ones = n_slots * smr_per
    smr_tot = smr_ones + PART

    key = (tuple(s_sizes), D, H, VARIANT)
    if key not in _cache:
        _cache[key] = _build_nc(s_sizes, D, H, VARIANT)
    nc = _cache[key]

    w1f = kd * H
    in_maps = []
    scatter = []  # (core, unit_idx, idx_global_rows)
    for c in range(N_CORES):
        buf = np.zeros((PART, ftot), dtype=np.float32)
        smr = np.zeros((1, smr_tot), dtype=np.float32)
        smr[0, smr_ones:smr_ones + PART] = 1.0
        in_off = 0
        for j, s in enumerate(s_sizes):
            r = j * N_CORES + c
            a = int(rank[r]) if r < A else -1
            if a >= 0:
                n_a = int(counts[a])
                idx = order[starts[a]:starts[a] + n_a]
                kh = kd // 2
                w1p = (W1_embs[a].reshape(kd, PART, H)
                       .transpose(1, 0, 2).reshape(PART, kd * H))
                if n_a > 0:
                    pidx = np.concatenate([idx, np.repeat(idx[:1], s - n_a)])
                    xtp = (X[pidx].T.reshape(kd, PART, s)
                           .transpose(1, 0, 2).reshape(PART, kd * s))
                else:
                    xtp = np.zeros((PART, kd * s), dtype=np.float32)
                for g in range(2):
                    base = in_off + g * fs[j]
                    buf[:, base:base + kh * H] = (
                        w1p[:, g * kh * H:(g + 1) * kh * H])
                    buf[:, base + offs[j]:base + offs[j] + kh * s] = (
                        xtp[:, g * kh * s:(g + 1) * kh * s])
                smr[0, j * smr_per:j * smr_per + H] = b1_embs[a]
                w2c = W2_embs[a].reshape(H, T)
                for t in range(T):
                    base = j * smr_per + H + t * (H + 2)
                    smr[0, base:base + H] = w2c[:, t]
                    smr[0, base + H] = b2_embs[a][t]
                for ui, (jj, s0, sc) in enumerate(units):
                    if jj == j and s0 < n_a:
                        scatter.append((c, ui, idx[s0:s0 + sc]))
            in_off += 2 * fs[j]
        in_maps.append({"IN": buf, "SMALLR": smr})

    if trace:
        _install_ntff_hook()
    from concourse import bass_utils
    bass_utils.upload_artifacts = lambda tmpdir: str(tmpdir)
    res = bass_utils.run_bass_kernel_spmd(
        nc, in_maps, list(range(N_CORES)), trace=trace)

    out = np.zeros((B, T), dtype=np.float32)
    for c, ui, idx in scatter:
        out[idx] = res.results[c]["OUT"][:len(idx), T * ui:T * ui + T]
    return out, res


def kernel(**inputs):
    out, _ = _run(**inputs)
    return out
